# revision 1
# baseline (speedup 1.0000x reference)
"""Compressed-KV GPT-2 attention block on 8 TRN2 NeuronCores.

Sharding: batch x head-group. Core c: batch b = c//4, heads 4*(c%4)..4*(c%4)+4.
Each core runs the full fused pipeline for its 4 heads in transposed-activation
layout ([dim, seq] on partitions) and emits a partial c_proj output^T; the host
sums the 4 partials per batch and adds b_proj.

Device pipeline per core (all matmuls bf16 -> fp32 PSUM):
  The KV compressor is low-rank and linear, so host folds it:
    k_dec = k @ (wk_c@wk_d)  -> fold W_k into w_attn k-columns (w_k' = w_k W_k)
    v_dec = v @ (wv_c@wv_d)  -> one small on-device matmul with W_v
  qkv^T   = w_qkv^T-chunks @ hidden^T   (m-blocks: q|q, k'|k', v|v head pairs,
            so kdec^T comes straight out of the qkv matmul)
  vdec    = v^T-slices^T @ W_v          (natural [s,d] + ones col for denom)
  S^T     = kdec^T-slices^T @ q^T   -> exp (no-max softmax; causal via mask mul)
  attn^T  = vdec_ones^T @ E (accum over key tiles; row 64 = softmax denom)
  out^T  += w_proj-rows^T @ attn^T  (partial over this core's heads)
"""

import sys

if "/opt/trn_rl_repo" not in sys.path:
    sys.path.insert(0, "/opt/trn_rl_repo")

import numpy as np
import ml_dtypes

BF16 = ml_dtypes.bfloat16

B, S, D = 2, 2048, 1024
H, hd, C = 16, 64, 32
NCORES = 8
HPC = 4            # heads per core
SB = 512           # free-dim block (PSUM bank / max moving cols)
NSB = S // SB      # 4 seq blocks of 512
NKT = S // 128     # 16 key tiles of 128
DC = D // 128      # 8 contraction chunks for qkv
PMB = D // 128     # 8 output-row blocks for c_proj

_cache = {}


def _build():
    import concourse.bacc as bacc
    import concourse.tile as tile
    import concourse.mybir as mybir

    dt = mybir.dt
    f32, bf16 = dt.float32, dt.bfloat16
    Exp = mybir.ActivationFunctionType.Exp
    mult = mybir.AluOpType.mult

    nc = bacc.Bacc("TRN2", target_bir_lowering=False, debug=False, num_devices=NCORES)

    hidden_t = nc.dram_tensor("hidden_t", [D, S], bf16, kind="ExternalInput")
    w_qkv = nc.dram_tensor("w_qkv", [D, 6 * 128], bf16, kind="ExternalInput")
    b_qkv = nc.dram_tensor("b_qkv", [128, 6], f32, kind="ExternalInput")
    wv = nc.dram_tensor("wv", [HPC, hd, hd], bf16, kind="ExternalInput")
    w_proj = nc.dram_tensor("w_proj", [HPC, hd, D], bf16, kind="ExternalInput")
    maskbig = nc.dram_tensor("maskbig", [128, 896], bf16, kind="ExternalInput")
    out_t = nc.dram_tensor("out_t", [D, S], bf16, kind="ExternalOutput")

    with tile.TileContext(nc) as tc:
        with (
            tc.tile_pool(name="persist", bufs=1) as pp,
            tc.tile_pool(name="work", bufs=4) as wp,
            tc.tile_pool(name="epool", bufs=36) as ep,
            tc.tile_pool(name="ostage", bufs=3) as op,
            tc.tile_pool(name="dscr", bufs=4, space="DRAM") as dr,
            tc.tile_pool(name="ps_big", bufs=5, space="PSUM") as ps_big,
            tc.tile_pool(name="ps_o", bufs=3, space="PSUM") as ps_o,
        ):
            # ---- load weights first, then hidden in consumption order ----
            bias = pp.tile([128, 6], f32, tag="bias", name="bias")
            nc.sync.dma_start(bias[:], b_qkv.ap())
            wq = []
            for d in range(DC):
                w = pp.tile([128, 6 * 128], bf16, tag=f"wq{d}", name=f"wq{d}")
                nc.sync.dma_start(w[:], w_qkv.ap()[d * 128:(d + 1) * 128, :])
                wq.append(w)
            hT = [pp.tile([128, S], bf16, tag=f"hT{d}", name=f"hT{d}") for d in range(DC)]
            for sb in range(NSB):
                for d in range(DC):
                    nc.sync.dma_start(
                        hT[d][:, sb * SB:(sb + 1) * SB],
                        hidden_t.ap()[d * 128:(d + 1) * 128, sb * SB:(sb + 1) * SB],
                    )
            maskt = pp.tile([128, 896], bf16, tag="mask", name="maskt")
            nc.sync.dma_start(maskt[:], maskbig.ap())

            wv_t, wpj = [], []
            for h in range(HPC):
                p = (h % 2) * 64
                t = pp.tile([128, hd], bf16, tag=f"wv{h}", name=f"wv{h}")
                nc.sync.dma_start(t[p:p + 64, :], wv.ap()[h])
                wv_t.append(t)
            for p in range(2):
                t = pp.tile([128, D], bf16, tag=f"wpj{p}", name=f"wpj{p}")
                nc.sync.dma_start(t[0:hd, :], w_proj.ap()[2 * p])
                nc.sync.dma_start(t[hd:128, :], w_proj.ap()[2 * p + 1])
                wpj.append(t)

            # ---- qkv^T: 6 m-blocks (q|q, k'|k', v|v head pairs) x 4 s-blocks ----
            qq = [pp.tile([128, S], bf16, tag=f"qq{p}", name=f"qq{p}") for p in range(2)]
            kk = [pp.tile([128, S], bf16, tag=f"kk{p}", name=f"kk{p}") for p in range(2)]
            vt = [pp.tile([128, S], bf16, tag=f"vt{p}", name=f"vt{p}") for p in range(2)]
            dests = qq + kk + vt
            for sb in range(NSB):
                for mb in range(6):
                    ps = ps_big.tile([128, SB], f32, tag="psS", name="psS")
                    for d in range(DC):
                        nc.tensor.matmul(
                            ps[:],
                            wq[d][:, mb * 128:(mb + 1) * 128],
                            hT[d][:, sb * SB:(sb + 1) * SB],
                            start=(d == 0),
                            stop=(d == DC - 1),
                        )
                    nc.vector.tensor_scalar_add(
                        out=dests[mb][:, sb * SB:(sb + 1) * SB],
                        in0=ps[:],
                        scalar1=bias[:, mb:mb + 1],
                    )

            def rows(h):
                p = (h % 2) * 64
                return slice(p, p + 64)

            def qT(h):
                return qq[h // 2][rows(h), :]

            def kdecT(h):
                return kk[h // 2][rows(h), :]

            def vT(h):
                return vt[h // 2][rows(h), :]

            # ---- per-head v decompress (W_v folded on host) + ones column ----
            vdo = [pp.tile([128, NKT * (hd + 1)], bf16, tag=f"vdo{h}", name=f"vdo{h}") for h in range(HPC)]
            for h in range(HPC):
                nc.vector.memset(vdo[h][:], 1.0)
                for st in range(NKT):
                    ps = ps_o.tile([128, hd], f32, tag="psO", name="psC")
                    nc.tensor.matmul(
                        ps[:],
                        vT(h)[:, st * 128:(st + 1) * 128],
                        wv_t[h][rows(h), :],
                    )
                    nc.vector.tensor_copy(
                        vdo[h][:, st * (hd + 1):st * (hd + 1) + hd], ps[:]
                    )

            # ---- attention + merge ----
            # attn packed in head pairs for K=128 c_proj: tile p rows 0-63 =
            # head 2p, rows 64-127 = head 2p+1 (odd heads via DMA shift)
            attn = [pp.tile([128, S], bf16, tag=f"attn{p}", name=f"attn{p}") for p in range(2)]
            for h in range(HPC):
                for qsb in range(NSB):
                    qsl = slice(qsb * SB, (qsb + 1) * SB)
                    nkb = 4 * qsb + 4
                    pso = ps_o.tile([hd + 1, SB], f32, tag="psO", name="psO")
                    es = []
                    for kb in range(nkb):
                        r = kb - 4 * qsb
                        c0 = max(r, 0) * 128  # cols < c0 are causally dead
                        psS = ps_big.tile([128, SB], f32, tag="psS", name="psS")
                        nc.tensor.matmul(
                            psS[:, c0:SB],
                            kdecT(h)[:, kb * 128:(kb + 1) * 128],
                            qT(h)[:, qsb * SB + c0:(qsb + 1) * SB],
                        )
                        e = ep.tile([128, SB], bf16, tag="E", name="e")
                        if r < 0:
                            nc.scalar.activation(e[:], psS[:], Exp)
                        else:
                            # band tile: one diagonal 128-col block, rest valid
                            c1 = c0 + 128
                            et = wp.tile([128, 128], bf16, tag="etd", name="etd")
                            nc.scalar.activation(et[:], psS[:, c0:c1], Exp)
                            nc.vector.tensor_tensor(
                                e[:, c0:c1], et[:], maskt[:, 384:512], mult
                            )
                            if c1 < SB:
                                nc.scalar.activation(e[:, c1:SB], psS[:, c1:SB], Exp)
                        es.append((e, c0))
                    for kb in range(nkb):
                        e, c0 = es[kb]
                        nc.tensor.matmul(
                            pso[:, c0:SB],
                            vdo[h][:, kb * (hd + 1):(kb + 1) * (hd + 1)],
                            e[:, c0:SB],
                            start=(kb == 0),
                            stop=(kb == nkb - 1),
                        )
                    # normalize: num/den via DMA-bounced denominator broadcast
                    nsb = wp.tile([hd + 1, SB], bf16, tag="nsb", name="nsb")
                    nc.vector.tensor_copy(nsb[:], pso[:])
                    den_d = dr.tile([SB], bf16, tag="den_d", name="den_d")
                    nc.sync.dma_start(den_d[:], nsb[hd:hd + 1, :])
                    den_c = wp.tile([128, 4], bf16, tag="den_c", name="den_c")
                    nc.sync.dma_start(
                        den_c[:], den_d[:].rearrange("(p j) -> p j", p=128)
                    )
                    rec_c = wp.tile([128, 4], bf16, tag="rec_c", name="rec_c")
                    with nc.allow_low_precision(reason="softmax denom recip in bf16"):
                        nc.vector.reciprocal(rec_c[:], den_c[:])
                    rec_d = dr.tile([SB], bf16, tag="rec_d", name="rec_d")
                    nc.sync.dma_start(
                        rec_d[:].rearrange("(p j) -> p j", p=128), rec_c[:]
                    )
                    bcast = wp.tile([hd, SB], bf16, tag="bcast", name="bcast")
                    nc.sync.dma_start(
                        bcast[:], rec_d[:].unsqueeze(0).to_broadcast([hd, SB])
                    )
                    if h % 2 == 0:
                        nc.vector.tensor_tensor(
                            attn[h // 2][0:hd, qsl], nsb[0:hd, :], bcast[:], mult
                        )
                    else:
                        # odd head lands on partitions 64-127: DVE can't cross
                        # partitions, so mul into a tmp then DMA-shift
                        atmp = wp.tile([hd, SB], bf16, tag="atmp", name="atmp")
                        nc.vector.tensor_tensor(atmp[:], nsb[0:hd, :], bcast[:], mult)
                        nc.sync.dma_start(attn[h // 2][hd:128, qsl], atmp[:])

            # ---- partial c_proj: out^T[mb*128:, sb*512:], K=128 per pair ----
            for sb in range(NSB):
                sl = slice(sb * SB, (sb + 1) * SB)
                for mb in range(PMB):
                    ps = ps_big.tile([128, SB], f32, tag="psS", name="psP")
                    for p in range(2):
                        nc.tensor.matmul(
                            ps[:],
                            wpj[p][:, mb * 128:(mb + 1) * 128],
                            attn[p][:, sl],
                            start=(p == 0),
                            stop=(p == 1),
                        )
                    stage = op.tile([128, SB], bf16, tag="stage", name="stage")
                    nc.vector.tensor_copy(stage[:], ps[:])
                    nc.sync.dma_start(out_t.ap()[mb * 128:(mb + 1) * 128, sl], stage[:])

    nc.compile()
    return nc


def _prep_inputs(hidden_states, w_attn, b_attn, wk_c, wv_c, wk_d, wv_d, w_proj):
    """Per-core input maps (host-side shard + pack + bf16 cast).

    The KV compressor is linear + low-rank, so it folds on host:
      W_k[h] = wk_c[h] @ wk_d[h] / sqrt(hd)  -> folded into w_attn k-columns
      W_v[h] = wv_c[h] @ wv_d[h]             -> single on-device matmul
    """
    hidden_T = [np.ascontiguousarray(hidden_states[b].T).astype(BF16) for b in range(B)]
    Wk = np.einsum("hdc,hce->hde", wk_c.astype(np.float64),
                   wk_d.astype(np.float64)) * (1.0 / np.sqrt(hd))  # [H,hd,hd]
    Wv = np.einsum("hdc,hce->hde", wv_c.astype(np.float64),
                   wv_d.astype(np.float64))                        # [H,hd,hd]
    wq_h = lambda h: w_attn[:, h * hd:(h + 1) * hd]
    wk_h = lambda h: (w_attn[:, D + h * hd:D + (h + 1) * hd].astype(np.float64)
                      @ Wk[h]).astype(np.float32)
    wv_h = lambda h: w_attn[:, 2 * D + h * hd:2 * D + (h + 1) * hd]
    bq_h = lambda h: b_attn[h * hd:(h + 1) * hd]
    bk_h = lambda h: (b_attn[D + h * hd:D + (h + 1) * hd].astype(np.float64)
                      @ Wk[h]).astype(np.float32)
    bv_h = lambda h: b_attn[2 * D + h * hd:2 * D + (h + 1) * hd]
    in_maps = []
    for c in range(NCORES):
        b = c // 4
        hs = list(range((c % 4) * HPC, (c % 4) * HPC + HPC))
        # m-blocks: [q0|q1], [q2|q3], [k'0|k'1], [k'2|k'3], [v0|v1], [v2|v3]
        cols, bcols = [], []
        for fn, bfn in ((wq_h, bq_h), (wk_h, bk_h), (wv_h, bv_h)):
            for h in hs:
                cols.append(fn(h))
                bcols.append(bfn(h))
        w_qkv_l = np.concatenate(cols, axis=1).astype(BF16)        # [1024, 768]
        b_qkv_l = (
            np.concatenate(bcols).astype(np.float32).reshape(6, 128).T.copy()
        )                                                          # [128, 6]
        k = np.arange(128).reshape(128, 1)
        cgrid = np.arange(896).reshape(1, 896)
        mask = (k <= cgrid - 384).astype(BF16)
        in_maps.append(
            {
                "hidden_t": hidden_T[b],
                "w_qkv": w_qkv_l,
                "b_qkv": b_qkv_l,
                "wv": Wv[hs].astype(BF16),
                "w_proj": np.stack(
                    [w_proj[h * hd:(h + 1) * hd, :] for h in hs]
                ).astype(BF16),
                "maskbig": np.ascontiguousarray(mask),
            }
        )
    return in_maps


def kernel(
    hidden_states,
    w_attn,
    b_attn,
    w_proj,
    b_proj,
    wk_c,
    wv_c,
    wk_d,
    wv_d,
    _trace=False,
):
    from concourse.bass_utils import run_bass_kernel_spmd

    if "nc" not in _cache:
        _cache["nc"] = _build()
    nc = _cache["nc"]

    in_maps = _prep_inputs(
        np.asarray(hidden_states),
        np.asarray(w_attn),
        np.asarray(b_attn),
        np.asarray(wk_c),
        np.asarray(wv_c),
        np.asarray(wk_d),
        np.asarray(wv_d),
        np.asarray(w_proj),
    )
    res = run_bass_kernel_spmd(
        nc, in_maps, core_ids=list(range(NCORES)), trace=_trace
    )
    out = np.empty((B, S, D), np.float32)
    for b in range(B):
        acc = np.zeros((D, S), np.float32)
        for c in range(4 * b, 4 * b + 4):
            acc += res.results[c]["out_t"].astype(np.float32)
        out[b] = acc.T + np.asarray(b_proj, np.float32)
    if _trace:
        _cache["last_exec_time_ns"] = res.exec_time_ns
        _cache["last_results"] = res
    return out



# revision 2
# speedup vs baseline: 1.3638x; 1.3638x over previous
"""Compressed-KV GPT-2 attention block on 8 TRN2 NeuronCores.

Sharding: batch x head-group. Core c: batch b = c//4, heads 4*(c%4)..+4.

The KV compressor is linear + low-rank, so both sides fold on host:
  scores = q @ (k_c wk_d)^T / 8 = (q wk_d^T / 8) @ k_c^T   -> q' [S,32]
  out_h  = (P @ v_c) @ (wv_d w_proj_h)                     -> attn in C=32 space
so the device pipeline works entirely in the compressed C=32 head space:
  qkv'  : hidden^T -> q'^T, k_c^T (32 rows/head), v^T (raw, 64 rows/head)
  v_c   : per key tile, v^T slices^T @ wv_c -> v_c [keys, 32/head] (64x128-mode
          row pairs)
  S^T   : k_c^T slices^T @ q'^T, 4 heads packed via 32x128-mode PE row tiling
          into one 4-bank PSUM tile; exp on ScalarE in one [128, ~2048] op
  attn  : v_c^T @ E + ones^T @ E (denominator), 4 heads packed via 128x32-mode
          PE column tiling; normalize by DMA-bounced reciprocal broadcast
  out^T : w_proj'^T-chunks @ attn_norm (K=128 covers all 4 heads at C=32)
Host sums the 4 partials per batch and adds b_proj.

The PE instruction stream hand-interleaves qkv/v_c/proj filler work into the
score/attnV stream so the PE tracks the ScalarE exp pace (the bottleneck).
"""

import sys

if "/opt/trn_rl_repo" not in sys.path:
    sys.path.insert(0, "/opt/trn_rl_repo")

import numpy as np
import ml_dtypes

BF16 = ml_dtypes.bfloat16

B, S, D = 2, 2048, 1024
H, hd, C = 16, 64, 32
NCORES = 8
HPC = 4            # heads per core
SB = 512           # q block
NSB = S // SB      # 4
KT = 128           # keys per tile
NKT = S // KT      # 16
DC = D // 128      # 8 contraction chunks

_cache = {}


def _build():
    import concourse.bacc as bacc
    import concourse.tile as tile
    import concourse.mybir as mybir

    dt = mybir.dt
    f32, bf16 = dt.float32, dt.bfloat16
    Exp = mybir.ActivationFunctionType.Exp
    mult = mybir.AluOpType.mult

    nc = bacc.Bacc("TRN2", target_bir_lowering=False, debug=False, num_devices=NCORES)

    hidden_t = nc.dram_tensor("hidden_t", [D, S], bf16, kind="ExternalInput")
    w_qkv = nc.dram_tensor("w_qkv", [D, 4 * 128], bf16, kind="ExternalInput")
    b_qkv = nc.dram_tensor("b_qkv", [128, 4], f32, kind="ExternalInput")
    wv_in = nc.dram_tensor("wv_in", [128, 2 * C], bf16, kind="ExternalInput")
    w_projp = nc.dram_tensor("w_projp", [128, D], bf16, kind="ExternalInput")
    mask_in = nc.dram_tensor("mask_in", [128, 4 * KT], bf16, kind="ExternalInput")
    out_t = nc.dram_tensor("out_t", [D, S], bf16, kind="ExternalOutput")

    with tile.TileContext(nc) as tc:
        with (
            tc.tile_pool(name="persist", bufs=1) as pp,
            tc.tile_pool(name="epool", bufs=6) as ep,
            tc.tile_pool(name="npool", bufs=2) as npo,
            tc.tile_pool(name="ostage", bufs=3) as op,
            tc.tile_pool(name="dscr", bufs=2, space="DRAM") as dr,
            tc.tile_pool(name="ps_sc", bufs=1, space="PSUM") as ps_sc,
            tc.tile_pool(name="ps_at", bufs=1, space="PSUM") as ps_at,
            tc.tile_pool(name="ps_dn", bufs=1, space="PSUM") as ps_dn,
            tc.tile_pool(name="ps_big", bufs=2, space="PSUM") as ps_big,
        ):
            # ---- weights, then hidden in consumption order ----
            bias = pp.tile([128, 4], f32, tag="bias", name="bias")
            nc.sync.dma_start(bias[:], b_qkv.ap())
            wq = []
            for d in range(DC):
                w = pp.tile([128, 4 * 128], bf16, tag=f"wq{d}", name=f"wq{d}")
                nc.sync.dma_start(w[:], w_qkv.ap()[d * 128:(d + 1) * 128, :])
                wq.append(w)
            wvt = pp.tile([128, 2 * C], bf16, tag="wvt", name="wvt")
            nc.sync.dma_start(wvt[:], wv_in.ap())
            mask4 = pp.tile([128, 4 * KT], bf16, tag="mask4", name="mask4")
            nc.sync.dma_start(mask4[:], mask_in.ap())
            hT = [pp.tile([128, S], bf16, tag=f"hT{d}", name=f"hT{d}") for d in range(DC)]
            for sb in range(NSB):
                for d in range(DC):
                    nc.sync.dma_start(
                        hT[d][:, sb * SB:(sb + 1) * SB],
                        hidden_t.ap()[d * 128:(d + 1) * 128, sb * SB:(sb + 1) * SB],
                    )
            wpj = pp.tile([128, D], bf16, tag="wpj", name="wpj")
            nc.sync.dma_start(wpj[:], w_projp.ap())

            ones = pp.tile([128, 1], bf16, tag="ones", name="ones")
            nc.vector.memset(ones[:], 1.0)
            # preload the Exp table set early (off the critical path)
            warm = pp.tile([128, 1], bf16, tag="warm", name="warm")
            nc.scalar.activation(warm[:], ones[:], Exp)

            # qkv'^T destinations
            qp = pp.tile([128, S], bf16, tag="qp", name="qp")
            kcT = pp.tile([128, S], bf16, tag="kcT", name="kcT")
            vT = [pp.tile([128, S], bf16, tag=f"vT{p}", name=f"vT{p}") for p in range(2)]
            dests = [qp, kcT, vT[0], vT[1]]
            vpack = [pp.tile([128, 128], bf16, tag=f"vpk{t}", name=f"vpk{t}")
                     for t in range(NKT)]

            def qkv_group(sb, mb):
                ps = ps_big.tile([128, SB], f32, tag="big", name="psQ")
                for d in range(DC):
                    nc.tensor.matmul(
                        ps[:],
                        wq[d][:, mb * 128:(mb + 1) * 128],
                        hT[d][:, sb * SB:(sb + 1) * SB],
                        start=(d == 0),
                        stop=(d == DC - 1),
                    )
                nc.vector.tensor_scalar_add(
                    out=dests[mb][:, sb * SB:(sb + 1) * SB],
                    in0=ps[:],
                    scalar1=bias[:, mb:mb + 1],
                )

            def vc_pair(kt, p):
                # heads 2p (rows 0-63) and 2p+1 (rows 64-127) of vT[p]
                psa = ps_big.tile([128, SB], f32, tag="big", name="psVa")
                psb = ps_big.tile([128, SB], f32, tag="big", name="psVb")
                nc.tensor.matmul(
                    psa[:, 0:C], vT[p][0:64, kt * KT:(kt + 1) * KT],
                    wvt[0:64, C * p:C * p + C], tile_position=(0, 0),
                )
                nc.tensor.matmul(
                    psb[:, 0:C], vT[p][64:128, kt * KT:(kt + 1) * KT],
                    wvt[64:128, C * p:C * p + C], tile_position=(64, 0),
                )
                nc.vector.tensor_copy(
                    vpack[kt][:, 64 * p:64 * p + C], psa[:, 0:C])
                nc.vector.tensor_copy(
                    vpack[kt][:, 64 * p + C:64 * p + 2 * C], psb[:, 0:C])

            es = {}

            def scores_group(qsb, kb):
                r = kb - 4 * qsb
                c0 = max(r, 0) * KT
                sc = ps_sc.tile([128, 4 * SB], f32, tag="sc", name="sc")
                for h in range(HPC):
                    nc.tensor.matmul(
                        sc[:, h * SB + c0:(h + 1) * SB],
                        kcT[32 * h:32 * h + 32, kb * KT:(kb + 1) * KT],
                        qp[32 * h:32 * h + 32, qsb * SB + c0:(qsb + 1) * SB],
                        tile_position=(32 * h, 0),
                    )
                e = ep.tile([128, 4 * SB], bf16, tag="e", name="e")
                scv = sc[:].rearrange("p (h w) -> p h w", h=4)[:, :, c0:SB]
                ev = e[:].rearrange("p (h w) -> p h w", h=4)[:, :, c0:SB]
                nc.scalar.activation(ev, scv, Exp)
                if r >= 0:
                    # mask the diagonal 128-col block of each head
                    ed = e[:].rearrange("p (h w) -> p h w", h=4)[:, :, c0:c0 + KT]
                    mv = mask4[:].rearrange("p (h w) -> p h w", h=4)
                    nc.vector.tensor_tensor(ed, ed, mv, mult)
                es[(qsb, kb)] = e

            attn_ps = {}
            den_ps = {}

            def attn_group(qsb, kb):
                r = kb - 4 * qsb
                c0 = max(r, 0) * KT
                nkb = 4 * qsb + 4
                if kb == 0:
                    attn_ps[qsb] = ps_at.tile([128, SB], f32, tag="at", name="at")
                    den_ps[qsb] = ps_dn.tile([128, SB], f32, tag="dn", name="dn")
                at, dn = attn_ps[qsb], den_ps[qsb]
                e = es.pop((qsb, kb))
                for h in range(HPC):
                    ee = e[:, h * SB + c0:(h + 1) * SB]
                    nc.tensor.matmul(
                        at[32 * h:32 * h + 32, c0:SB], vpack[kb][:, 32 * h:32 * h + 32],
                        ee, tile_position=(0, 32 * h),
                        start=(kb == 0), stop=(kb == nkb - 1),
                    )
                    nc.tensor.matmul(
                        dn[32 * h:32 * h + 1, c0:SB], ones[:, 0:1],
                        ee, tile_position=(0, 32 * h),
                        start=(kb == 0), stop=(kb == nkb - 1),
                    )

            attn_norm = {}

            def norm(qsb):
                at = attn_ps.pop(qsb)
                dn = den_ps.pop(qsb)
                attn_sb = npo.tile([128, SB], bf16, tag="attn_sb", name="attn_sb")
                nc.vector.tensor_copy(attn_sb[:], at[:])
                den_sb = npo.tile([128, SB], bf16, tag="den_sb", name="den_sb")
                nc.vector.tensor_copy(den_sb[:], dn[:])
                rec_sb = npo.tile([128, SB], bf16, tag="rec_sb", name="rec_sb")
                with nc.allow_low_precision(reason="softmax denom recip in bf16"):
                    nc.vector.reciprocal(rec_sb[:], den_sb[:])
                rec_dr = dr.tile([HPC, SB], bf16, tag="rec_dr", name="rec_dr")
                nc.sync.dma_start(
                    rec_dr[:], rec_sb[:].rearrange("(h s) q -> h s q", s=32)[:, 0, :]
                )
                recb = npo.tile([128, SB], bf16, tag="recb", name="recb")
                for h in range(HPC):
                    nc.sync.dma_start(
                        recb[32 * h:32 * h + 32, :],
                        rec_dr[h:h + 1, :].to_broadcast([32, SB]),
                    )
                an = npo.tile([128, SB], bf16, tag="an", name="an")
                nc.vector.tensor_tensor(an[:], attn_sb[:], recb[:], mult)
                attn_norm[qsb] = an

            def proj_group(qsb, mb):
                ps = ps_big.tile([128, SB], f32, tag="big", name="psP")
                nc.tensor.matmul(
                    ps[:], wpj[:, mb * 128:(mb + 1) * 128], attn_norm[qsb][:],
                )
                stage = op.tile([128, SB], bf16, tag="stage", name="stage")
                nc.vector.tensor_copy(stage[:], ps[:])
                nc.sync.dma_start(
                    out_t.ap()[mb * 128:(mb + 1) * 128, qsb * SB:(qsb + 1) * SB],
                    stage[:],
                )

            # ================= PE schedule =================
            # lead-in: qkv sb0, v_c kt0-3
            for mb in range(4):
                qkv_group(0, mb)
            for kt in range(4):
                vc_pair(kt, 0)
                vc_pair(kt, 1)

            # qsb0 (4 kb) with qkv sb1 fillers
            scores_group(0, 0)
            qkv_group(1, 0)
            scores_group(0, 1); attn_group(0, 0)
            qkv_group(1, 1)
            scores_group(0, 2); attn_group(0, 1)
            qkv_group(1, 2)
            scores_group(0, 3); attn_group(0, 2)
            qkv_group(1, 3)
            attn_group(0, 3)
            norm(0)
            for kt in range(4, 8):
                vc_pair(kt, 0)
                vc_pair(kt, 1)

            # qsb1 (8 kb) with qkv sb2/sb3 fillers
            fillers = [(2, 0), (2, 1), (2, 2), (2, 3), (3, 0), (3, 1), (3, 2), (3, 3)]
            for kb in range(8):
                scores_group(1, kb)
                if kb >= 1:
                    attn_group(1, kb - 1)
                qkv_group(*fillers[kb])
            attn_group(1, 7)
            norm(1)
            for kt in range(8, 16):
                vc_pair(kt, 0)
                vc_pair(kt, 1)

            # qsb2 (12 kb) with proj(0)/proj(1) fillers
            for kb in range(12):
                scores_group(2, kb)
                if kb >= 1:
                    attn_group(2, kb - 1)
                if kb < 8:
                    proj_group(0, kb)
            attn_group(2, 11)
            norm(2)

            # qsb3 (16 kb) with proj(1)/proj(2) fillers
            for kb in range(16):
                scores_group(3, kb)
                if kb >= 1:
                    attn_group(3, kb - 1)
                if kb < 8:
                    proj_group(1, kb)
                elif kb < 16:
                    proj_group(2, kb - 8)
            attn_group(3, 15)
            norm(3)
            for mb in range(8):
                proj_group(3, mb)

    nc.compile()
    return nc


def _prep_inputs(hidden_states, w_attn, b_attn, wk_c, wv_c, wk_d, wv_d, w_proj):
    """Per-core input maps (host-side shard + fold + pack + bf16 cast)."""
    f8 = np.float64
    hidden_T = [np.ascontiguousarray(hidden_states[b].T).astype(BF16) for b in range(B)]
    w_attn8, b_attn8 = w_attn.astype(f8), b_attn.astype(f8)
    scl = 1.0 / np.sqrt(hd)

    k = np.arange(128).reshape(128, 1)
    j = np.arange(KT).reshape(1, KT)
    mask1 = (k <= j).astype(BF16)
    mask4 = np.tile(mask1, (1, 4))

    in_maps = []
    for c in range(NCORES):
        b = c // 4
        hs = [4 * (c % 4) + h for h in range(HPC)]
        wq_cols, bq = [], []
        wk_cols, bk = [], []
        for h in hs:
            wq_cols.append(w_attn8[:, h * hd:(h + 1) * hd] @ wk_d[h].astype(f8).T * scl)
            bq.append(b_attn8[h * hd:(h + 1) * hd] @ wk_d[h].astype(f8).T * scl)
            wk_cols.append(w_attn8[:, D + h * hd:D + (h + 1) * hd] @ wk_c[h].astype(f8))
            bk.append(b_attn8[D + h * hd:D + (h + 1) * hd] @ wk_c[h].astype(f8))
        wv_cols = [w_attn[:, 2 * D + h * hd:2 * D + (h + 1) * hd] for h in hs]
        bv = [b_attn[2 * D + h * hd:2 * D + (h + 1) * hd] for h in hs]
        w_qkv_l = np.concatenate(
            [np.concatenate(wq_cols, 1), np.concatenate(wk_cols, 1),
             np.concatenate(wv_cols[0:2], 1), np.concatenate(wv_cols[2:4], 1)],
            axis=1,
        ).astype(BF16)                                          # [1024, 512]
        b_qkv_l = np.stack(
            [np.concatenate(bq), np.concatenate(bk),
             np.concatenate(bv[0:2]), np.concatenate(bv[2:4])],
            axis=1,
        ).astype(np.float32)                                    # [128, 4]
        wv_l = np.zeros((128, 2 * C), BF16)
        for p in range(2):
            wv_l[0:64, C * p:C * p + C] = wv_c[hs[2 * p]].astype(BF16)
            wv_l[64:128, C * p:C * p + C] = wv_c[hs[2 * p + 1]].astype(BF16)
        w_projp_l = np.concatenate(
            [wv_d[h].astype(f8) @ w_proj[h * hd:(h + 1) * hd, :].astype(f8)
             for h in hs], axis=0,
        ).astype(BF16)                                          # [128, 1024]
        in_maps.append(
            {
                "hidden_t": hidden_T[b],
                "w_qkv": w_qkv_l,
                "b_qkv": b_qkv_l,
                "wv_in": wv_l,
                "w_projp": w_projp_l,
                "mask_in": mask4,
            }
        )
    return in_maps


def kernel(
    hidden_states,
    w_attn,
    b_attn,
    w_proj,
    b_proj,
    wk_c,
    wv_c,
    wk_d,
    wv_d,
    _trace=False,
):
    from concourse.bass_utils import run_bass_kernel_spmd

    if "nc" not in _cache:
        _cache["nc"] = _build()
    nc = _cache["nc"]

    in_maps = _prep_inputs(
        np.asarray(hidden_states),
        np.asarray(w_attn),
        np.asarray(b_attn),
        np.asarray(wk_c),
        np.asarray(wv_c),
        np.asarray(wk_d),
        np.asarray(wv_d),
        np.asarray(w_proj),
    )
    res = run_bass_kernel_spmd(
        nc, in_maps, core_ids=list(range(NCORES)), trace=_trace
    )
    out = np.empty((B, S, D), np.float32)
    for b in range(B):
        acc = np.zeros((D, S), np.float32)
        for c in range(4 * b, 4 * b + 4):
            acc += res.results[c]["out_t"].astype(np.float32)
        out[b] = acc.T + np.asarray(b_proj, np.float32)
    if _trace:
        _cache["last_exec_time_ns"] = res.exec_time_ns
        _cache["last_results"] = res
    return out


# revision 9
# speedup vs baseline: 1.5418x; 1.1305x over previous
"""Compressed-KV GPT-2 attention block on 8 TRN2 NeuronCores.

Sharding: batch x head-group. Core c: batch b = c//4, heads 4*(c%4)..+4.

The KV compressor is linear + low-rank, so both sides fold on host:
  scores = q @ (k_c wk_d)^T / 8 = (q wk_d^T / 8) @ k_c^T   -> q' [S,32]
  out_h  = (P @ v_c) @ (wv_d w_proj_h)                     -> attn in C=32 space
so the device pipeline works entirely in the compressed C=32 head space:
  qkv'  : hidden^T -> q'^T, k_c^T (32 rows/head), v^T (raw, 64 rows/head)
  v_c   : per key tile, v^T slices^T @ wv_c -> v_c [keys, 32/head] (64x128-mode
          row pairs)
  S^T   : k_c^T slices^T @ q'^T, 4 heads packed via 32x128-mode PE row tiling
          into one 4-bank PSUM tile; exp on ScalarE in one [128, ~2048] op
  attn  : v_c^T @ E + ones^T @ E (denominator), 4 heads packed via 128x32-mode
          PE column tiling; normalize by DMA-bounced reciprocal broadcast
  out^T : w_proj'^T-chunks @ attn_norm (K=128 covers all 4 heads at C=32)
Host sums the 4 partials per batch and adds b_proj.

The PE instruction stream hand-interleaves qkv/v_c/proj filler work into the
score/attnV stream so the PE tracks the ScalarE exp pace (the bottleneck).
"""

import sys

if "/opt/trn_rl_repo" not in sys.path:
    sys.path.insert(0, "/opt/trn_rl_repo")

import numpy as np
import ml_dtypes

BF16 = ml_dtypes.bfloat16

B, S, D = 2, 2048, 1024
H, hd, C = 16, 64, 32
NCORES = 8
HPC = 4            # heads per core
SB = 512           # q block
NSB = S // SB      # 4
KT = 128           # keys per tile
NKT = S // KT      # 16
DC = D // 128      # 8 contraction chunks

_cache = {}


def _build():
    import concourse.bacc as bacc
    import concourse.tile as tile
    import concourse.mybir as mybir

    dt = mybir.dt
    f32, bf16 = dt.float32, dt.bfloat16
    Exp = mybir.ActivationFunctionType.Exp
    mult = mybir.AluOpType.mult

    nc = bacc.Bacc("TRN2", target_bir_lowering=False, debug=False, num_devices=NCORES)

    hidden_t = nc.dram_tensor("hidden_t", [D, S], bf16, kind="ExternalInput")
    w_qkv = nc.dram_tensor("w_qkv", [D, 4 * 128], bf16, kind="ExternalInput")
    b_qkv = nc.dram_tensor("b_qkv", [128, 4], f32, kind="ExternalInput")
    wv_in = nc.dram_tensor("wv_in", [128, 2 * C], bf16, kind="ExternalInput")
    w_projp = nc.dram_tensor("w_projp", [128, D], bf16, kind="ExternalInput")
    mask_in = nc.dram_tensor("mask_in", [128, 4 * KT], bf16, kind="ExternalInput")
    out_t = nc.dram_tensor("out_t", [D, S], bf16, kind="ExternalOutput")

    with tile.TileContext(nc) as tc:
        with (
            tc.tile_pool(name="persist", bufs=1) as pp,
            tc.tile_pool(name="epool", bufs=6) as ep,
            tc.tile_pool(name="npool", bufs=2) as npo,
            tc.tile_pool(name="ostage", bufs=3) as op,
            tc.tile_pool(name="dscr", bufs=2, space="DRAM") as dr,
            tc.tile_pool(name="ps_sc", bufs=1, space="PSUM") as ps_sc,
            tc.tile_pool(name="ps_at", bufs=1, space="PSUM") as ps_at,
            tc.tile_pool(name="ps_dn", bufs=1, space="PSUM") as ps_dn,
            tc.tile_pool(name="ps_big", bufs=2, space="PSUM") as ps_big,
        ):
            # ---- weights, then hidden in consumption order ----
            bias = pp.tile([128, 4], f32, tag="bias", name="bias")
            nc.sync.dma_start(bias[:], b_qkv.ap())
            wq = []
            for d in range(DC):
                w = pp.tile([128, 4 * 128], bf16, tag=f"wq{d}", name=f"wq{d}")
                nc.sync.dma_start(w[:], w_qkv.ap()[d * 128:(d + 1) * 128, :])
                wq.append(w)
            wvt = pp.tile([128, 2 * C], bf16, tag="wvt", name="wvt")
            nc.sync.dma_start(wvt[:], wv_in.ap())
            mask4 = pp.tile([128, 4 * KT], bf16, tag="mask4", name="mask4")
            nc.sync.dma_start(mask4[:], mask_in.ap())
            hT = [pp.tile([128, S], bf16, tag=f"hT{d}", name=f"hT{d}") for d in range(DC)]
            for sb in range(NSB):
                for d in range(DC):
                    nc.sync.dma_start(
                        hT[d][:, sb * SB:(sb + 1) * SB],
                        hidden_t.ap()[d * 128:(d + 1) * 128, sb * SB:(sb + 1) * SB],
                    )
            wpj = pp.tile([128, D], bf16, tag="wpj", name="wpj")
            nc.sync.dma_start(wpj[:], w_projp.ap())

            ones = pp.tile([128, 1], bf16, tag="ones", name="ones")
            nc.vector.memset(ones[:], 1.0)
            # preload the Exp table set early (off the critical path)
            warm = pp.tile([128, 1], bf16, tag="warm", name="warm")
            nc.scalar.activation(warm[:], ones[:], Exp)

            # qkv'^T destinations
            qp = pp.tile([128, S], bf16, tag="qp", name="qp")
            kcT = pp.tile([128, S], bf16, tag="kcT", name="kcT")
            vT = [pp.tile([128, S], bf16, tag=f"vT{p}", name=f"vT{p}") for p in range(2)]
            dests = [qp, kcT, vT[0], vT[1]]
            vpack = [pp.tile([128, 128], bf16, tag=f"vpk{t}", name=f"vpk{t}")
                     for t in range(NKT)]

            def qkv_group(sb, mb):
                ps = ps_big.tile([128, SB], f32, tag="big", name="psQ")
                for d in range(DC):
                    nc.tensor.matmul(
                        ps[:],
                        wq[d][:, mb * 128:(mb + 1) * 128],
                        hT[d][:, sb * SB:(sb + 1) * SB],
                        start=(d == 0),
                        stop=(d == DC - 1),
                    )
                nc.vector.tensor_scalar_add(
                    out=dests[mb][:, sb * SB:(sb + 1) * SB],
                    in0=ps[:],
                    scalar1=bias[:, mb:mb + 1],
                )

            def vc_pair(kt, p):
                # heads 2p (rows 0-63) and 2p+1 (rows 64-127) of vT[p]
                psa = ps_big.tile([128, SB], f32, tag="big", name="psVa")
                psb = ps_big.tile([128, SB], f32, tag="big", name="psVb")
                nc.tensor.matmul(
                    psa[:, 0:C], vT[p][0:64, kt * KT:(kt + 1) * KT],
                    wvt[0:64, C * p:C * p + C], tile_position=(0, 0),
                )
                nc.tensor.matmul(
                    psb[:, 0:C], vT[p][64:128, kt * KT:(kt + 1) * KT],
                    wvt[64:128, C * p:C * p + C], tile_position=(64, 0),
                )
                nc.vector.tensor_copy(
                    vpack[kt][:, 64 * p:64 * p + C], psa[:, 0:C])
                nc.vector.tensor_copy(
                    vpack[kt][:, 64 * p + C:64 * p + 2 * C], psb[:, 0:C])

            es = {}

            def scores_group(qsb, kb):
                r = kb - 4 * qsb
                c0 = max(r, 0) * KT
                sc = ps_sc.tile([128, 4 * SB], f32, tag="sc", name="sc")
                for h in range(HPC):
                    nc.tensor.matmul(
                        sc[:, h * SB + c0:(h + 1) * SB],
                        kcT[32 * h:32 * h + 32, kb * KT:(kb + 1) * KT],
                        qp[32 * h:32 * h + 32, qsb * SB + c0:(qsb + 1) * SB],
                        tile_position=(32 * h, 0),
                    )
                e = ep.tile([128, 4 * SB], bf16, tag="e", name="e")
                scv = sc[:].rearrange("p (h w) -> p h w", h=4)[:, :, c0:SB]
                ev = e[:].rearrange("p (h w) -> p h w", h=4)[:, :, c0:SB]
                nc.scalar.activation(ev, scv, Exp)
                if r >= 0:
                    # mask the diagonal 128-col block of each head
                    ed = e[:].rearrange("p (h w) -> p h w", h=4)[:, :, c0:c0 + KT]
                    mv = mask4[:].rearrange("p (h w) -> p h w", h=4)
                    nc.vector.tensor_tensor(ed, ed, mv, mult)
                es[(qsb, kb)] = e

            attn_ps = {}
            den_ps = {}

            def attn_group(qsb, kb):
                r = kb - 4 * qsb
                c0 = max(r, 0) * KT
                nkb = 4 * qsb + 4
                if kb == 0:
                    attn_ps[qsb] = ps_at.tile([128, SB], f32, tag="at", name="at")
                    den_ps[qsb] = ps_dn.tile([128, SB], f32, tag="dn", name="dn")
                at, dn = attn_ps[qsb], den_ps[qsb]
                e = es.pop((qsb, kb))
                for h in range(HPC):
                    ee = e[:, h * SB + c0:(h + 1) * SB]
                    nc.tensor.matmul(
                        at[32 * h:32 * h + 32, c0:SB], vpack[kb][:, 32 * h:32 * h + 32],
                        ee, tile_position=(0, 32 * h),
                        start=(kb == 0), stop=(kb == nkb - 1),
                    )
                    nc.tensor.matmul(
                        dn[32 * h:32 * h + 1, c0:SB], ones[:, 0:1],
                        ee, tile_position=(0, 32 * h),
                        start=(kb == 0), stop=(kb == nkb - 1),
                    )

            attn_norm = {}

            def norm(qsb):
                at = attn_ps.pop(qsb)
                dn = den_ps.pop(qsb)
                attn_sb = npo.tile([128, SB], bf16, tag="attn_sb", name="attn_sb")
                nc.vector.tensor_copy(attn_sb[:], at[:])
                den_sb = npo.tile([128, SB], bf16, tag="den_sb", name="den_sb")
                nc.vector.tensor_copy(den_sb[:], dn[:])
                # bounce den rows {32h} -> [128,16] for a cheap DVE reciprocal
                den_dr = dr.tile([HPC, SB], bf16, tag="den_dr", name="den_dr")
                nc.sync.dma_start(
                    den_dr[:], den_sb[:].rearrange("(h s) q -> h s q", s=32)[:, 0, :]
                )
                den_c = npo.tile([128, 16], bf16, tag="den_c", name="den_c")
                nc.sync.dma_start(
                    den_c[:], den_dr[:].rearrange("h (p j) -> (h p) j", j=16)
                )
                rec_c = npo.tile([128, 16], bf16, tag="rec_c", name="rec_c")
                with nc.allow_low_precision(reason="softmax denom recip in bf16"):
                    nc.vector.reciprocal(rec_c[:], den_c[:])
                rec_dr = dr.tile([HPC, SB], bf16, tag="rec_dr", name="rec_dr")
                nc.sync.dma_start(
                    rec_dr[:].rearrange("h (p j) -> (h p) j", j=16), rec_c[:]
                )
                recb = npo.tile([128, SB], bf16, tag="recb", name="recb")
                for h in range(HPC):
                    nc.sync.dma_start(
                        recb[32 * h:32 * h + 32, :],
                        rec_dr[h:h + 1, :].to_broadcast([32, SB]),
                    )
                an = npo.tile([128, SB], bf16, tag="an", name="an")
                nc.vector.tensor_tensor(an[:], attn_sb[:], recb[:], mult)
                attn_norm[qsb] = an

            def proj_group(qsb, mb):
                ps = ps_big.tile([128, SB], f32, tag="big", name="psP")
                nc.tensor.matmul(
                    ps[:], wpj[:, mb * 128:(mb + 1) * 128], attn_norm[qsb][:],
                )
                stage = op.tile([128, SB], bf16, tag="stage", name="stage")
                nc.vector.tensor_copy(stage[:], ps[:])
                nc.sync.dma_start(
                    out_t.ap()[mb * 128:(mb + 1) * 128, qsb * SB:(qsb + 1) * SB],
                    stage[:],
                )

            # ================= PE schedule =================
            # Filler work (qkv of later s-blocks, v_c compress, c_proj of
            # earlier q-blocks) is spread just-in-time across the qsb windows
            # so the PE stays dense (HAM-warm) while pacing the ScalarE exps.
            def QKV(sb, mb):
                return lambda: qkv_group(sb, mb)

            def VC(kt):
                def f():
                    vc_pair(kt, 0)
                    vc_pair(kt, 1)
                return f

            def PRJ(qsb, mb):
                return lambda: proj_group(qsb, mb)

            fillers = {
                0: [[QKV(1, 0), VC(2)], [QKV(1, 1), VC(3)],
                    [QKV(1, 2)], [QKV(1, 3)]],
                1: [[QKV(2, 0)], [QKV(2, 1)], [QKV(2, 2)],
                    [QKV(2, 3), VC(4)], [VC(5)], [VC(6)], [VC(7)], []],
                2: [[QKV(3, 0)], [QKV(3, 1)], [QKV(3, 2)], [QKV(3, 3)],
                    [PRJ(0, 0)], [PRJ(0, 1), VC(8)], [PRJ(0, 2), VC(9)],
                    [PRJ(0, 3), VC(10)], [PRJ(0, 4), VC(11)],
                    [PRJ(0, 5)], [PRJ(0, 6)], [PRJ(0, 7)]],
                3: [[PRJ(1, 0)], [PRJ(1, 1)], [PRJ(1, 2)], [PRJ(1, 3)],
                    [PRJ(1, 4)], [PRJ(1, 5)], [PRJ(1, 6)], [PRJ(1, 7)],
                    [PRJ(2, 0), VC(12)], [PRJ(2, 1), VC(13)],
                    [PRJ(2, 2), VC(14)], [PRJ(2, 3), VC(15)],
                    [PRJ(2, 4)], [PRJ(2, 5)], [PRJ(2, 6)], [PRJ(2, 7)]],
            }

            def run_filler(slot):
                for thunk in slot:
                    thunk()

            # lead-in: q'/k_c of sb0, first scores ASAP, then v of sb0
            qkv_group(0, 0)
            qkv_group(0, 1)
            scores_group(0, 0)
            qkv_group(0, 2)
            qkv_group(0, 3)
            vc_pair(0, 0)
            vc_pair(0, 1)
            vc_pair(1, 0)
            vc_pair(1, 1)

            for qsb in range(NSB):
                nkb = 4 * qsb + 4
                fl = fillers[qsb]
                for kb in range(nkb):
                    if not (qsb == 0 and kb == 0):
                        scores_group(qsb, kb)
                    if kb >= 1:
                        attn_group(qsb, kb - 1)
                    if kb < len(fl):
                        run_filler(fl[kb])
                attn_group(qsb, nkb - 1)
                norm(qsb)
            for mb in range(8):
                proj_group(3, mb)

    nc.compile()
    return nc


def _prep_inputs(hidden_states, w_attn, b_attn, wk_c, wv_c, wk_d, wv_d, w_proj):
    """Per-core input maps (host-side shard + fold + pack + bf16 cast)."""
    f8 = np.float64
    hidden_T = [np.ascontiguousarray(hidden_states[b].T).astype(BF16) for b in range(B)]
    w_attn8, b_attn8 = w_attn.astype(f8), b_attn.astype(f8)
    scl = 1.0 / np.sqrt(hd)

    k = np.arange(128).reshape(128, 1)
    j = np.arange(KT).reshape(1, KT)
    mask1 = (k <= j).astype(BF16)
    mask4 = np.tile(mask1, (1, 4))

    in_maps = []
    for c in range(NCORES):
        b = c // 4
        hs = [4 * (c % 4) + h for h in range(HPC)]
        wq_cols, bq = [], []
        wk_cols, bk = [], []
        for h in hs:
            wq_cols.append(w_attn8[:, h * hd:(h + 1) * hd] @ wk_d[h].astype(f8).T * scl)
            bq.append(b_attn8[h * hd:(h + 1) * hd] @ wk_d[h].astype(f8).T * scl)
            wk_cols.append(w_attn8[:, D + h * hd:D + (h + 1) * hd] @ wk_c[h].astype(f8))
            bk.append(b_attn8[D + h * hd:D + (h + 1) * hd] @ wk_c[h].astype(f8))
        wv_cols = [w_attn[:, 2 * D + h * hd:2 * D + (h + 1) * hd] for h in hs]
        bv = [b_attn[2 * D + h * hd:2 * D + (h + 1) * hd] for h in hs]
        w_qkv_l = np.concatenate(
            [np.concatenate(wq_cols, 1), np.concatenate(wk_cols, 1),
             np.concatenate(wv_cols[0:2], 1), np.concatenate(wv_cols[2:4], 1)],
            axis=1,
        ).astype(BF16)                                          # [1024, 512]
        b_qkv_l = np.stack(
            [np.concatenate(bq), np.concatenate(bk),
             np.concatenate(bv[0:2]), np.concatenate(bv[2:4])],
            axis=1,
        ).astype(np.float32)                                    # [128, 4]
        wv_l = np.zeros((128, 2 * C), BF16)
        for p in range(2):
            wv_l[0:64, C * p:C * p + C] = wv_c[hs[2 * p]].astype(BF16)
            wv_l[64:128, C * p:C * p + C] = wv_c[hs[2 * p + 1]].astype(BF16)
        w_projp_l = np.concatenate(
            [wv_d[h].astype(f8) @ w_proj[h * hd:(h + 1) * hd, :].astype(f8)
             for h in hs], axis=0,
        ).astype(BF16)                                          # [128, 1024]
        in_maps.append(
            {
                "hidden_t": hidden_T[b],
                "w_qkv": w_qkv_l,
                "b_qkv": b_qkv_l,
                "wv_in": wv_l,
                "w_projp": w_projp_l,
                "mask_in": mask4,
            }
        )
    return in_maps


def kernel(
    hidden_states,
    w_attn,
    b_attn,
    w_proj,
    b_proj,
    wk_c,
    wv_c,
    wk_d,
    wv_d,
    _trace=False,
):
    from concourse.bass_utils import run_bass_kernel_spmd

    if "nc" not in _cache:
        _cache["nc"] = _build()
    nc = _cache["nc"]

    in_maps = _prep_inputs(
        np.asarray(hidden_states),
        np.asarray(w_attn),
        np.asarray(b_attn),
        np.asarray(wk_c),
        np.asarray(wv_c),
        np.asarray(wk_d),
        np.asarray(wv_d),
        np.asarray(w_proj),
    )
    res = run_bass_kernel_spmd(
        nc, in_maps, core_ids=list(range(NCORES)), trace=_trace
    )
    out = np.empty((B, S, D), np.float32)
    for b in range(B):
        acc = np.zeros((D, S), np.float32)
        for c in range(4 * b, 4 * b + 4):
            acc += res.results[c]["out_t"].astype(np.float32)
        out[b] = acc.T + np.asarray(b_proj, np.float32)
    if _trace:
        _cache["last_exec_time_ns"] = res.exec_time_ns
        _cache["last_results"] = res
    return out


# revision 15
# speedup vs baseline: 1.6606x; 1.0771x over previous
"""Compressed-KV GPT-2 attention block on 8 TRN2 NeuronCores.

Sharding: batch x head-group. Core c: batch b = c//4, heads 4*(c%4)..+4.

The KV compressor is linear + low-rank, so both sides fold on host:
  scores = q @ (k_c wk_d)^T / 8 = (q wk_d^T / 8) @ k_c^T   -> q' [S,32]
  out_h  = (P @ v_c) @ (wv_d w_proj_h)                     -> attn in C=32 space
so the device pipeline works entirely in the compressed C=32 head space:
  qkv'  : hidden^T -> q'^T, k_c^T (32 rows/head), v^T (raw, 64 rows/head)
  v_c   : per key tile, v^T slices^T @ wv_c -> v_c [keys, 32/head] (64x128-mode
          row pairs)
  S^T   : k_c^T slices^T @ q'^T, 4 heads packed via 32x128-mode PE row tiling
          into one 4-bank PSUM tile; exp on ScalarE in one [128, ~2048] op
  attn  : v_c^T @ E + ones^T @ E (denominator), 4 heads packed via 128x32-mode
          PE column tiling; normalize by DMA-bounced reciprocal broadcast
  out^T : w_proj'^T-chunks @ attn_norm (K=128 covers all 4 heads at C=32)
Host sums the 4 partials per batch and adds b_proj.

The PE instruction stream hand-interleaves qkv/v_c/proj filler work into the
score/attnV stream so the PE tracks the ScalarE exp pace (the bottleneck).
"""

import sys

if "/opt/trn_rl_repo" not in sys.path:
    sys.path.insert(0, "/opt/trn_rl_repo")

import numpy as np
import ml_dtypes

BF16 = ml_dtypes.bfloat16

B, S, D = 2, 2048, 1024
H, hd, C = 16, 64, 32
NCORES = 8
HPC = 4            # heads per core
SB = 512           # q block
NSB = S // SB      # 4
KT = 128           # keys per tile
NKT = S // KT      # 16
DC = D // 128      # 8 contraction chunks

_cache = {}


def _build():
    import concourse.bacc as bacc
    import concourse.tile as tile
    import concourse.mybir as mybir

    dt = mybir.dt
    f32, bf16 = dt.float32, dt.bfloat16
    Exp = mybir.ActivationFunctionType.Exp
    mult = mybir.AluOpType.mult

    nc = bacc.Bacc("TRN2", target_bir_lowering=False, debug=False, num_devices=NCORES)

    hidden_t = nc.dram_tensor("hidden_t", [D, S], bf16, kind="ExternalInput")
    w_qkv = nc.dram_tensor("w_qkv", [D, 4 * 128], bf16, kind="ExternalInput")
    b_qkv = nc.dram_tensor("b_qkv", [128, 4], f32, kind="ExternalInput")
    wv_in = nc.dram_tensor("wv_in", [128, 2 * C], bf16, kind="ExternalInput")
    w_projp = nc.dram_tensor("w_projp", [128, D], bf16, kind="ExternalInput")
    mask_in = nc.dram_tensor("mask_in", [128, 4 * KT], bf16, kind="ExternalInput")
    out_t = nc.dram_tensor("out_t", [D, S], bf16, kind="ExternalOutput")

    with tile.TileContext(nc) as tc:
        with (
            tc.tile_pool(name="persist", bufs=1) as pp,
            tc.tile_pool(name="epool", bufs=6) as ep,
            tc.tile_pool(name="npool", bufs=2) as npo,
            tc.tile_pool(name="ostage", bufs=3) as op,
            tc.tile_pool(name="dscr", bufs=2, space="DRAM") as dr,
            tc.tile_pool(name="ps_scA", bufs=1, space="PSUM") as ps_scA,
            tc.tile_pool(name="ps_scB", bufs=1, space="PSUM") as ps_scB,
            tc.tile_pool(name="ps_at", bufs=1, space="PSUM") as ps_at,
            tc.tile_pool(name="ps_dn", bufs=1, space="PSUM") as ps_dn,
            tc.tile_pool(name="ps_big", bufs=2, space="PSUM") as ps_big,
        ):
            # ---- weights, then hidden in consumption order (few big DMAs) ----
            wq_all = pp.tile([128, DC * 4 * 128], bf16, tag="wq", name="wq_all")
            nc.sync.dma_start(
                wq_all[:].rearrange("p (d c) -> p d c", d=DC),
                w_qkv.ap().rearrange("(d p) c -> p d c", d=DC),
            )
            hT_all = pp.tile([128, DC * S], bf16, tag="hT", name="hT_all")
            nc.sync.dma_start(
                hT_all[:].rearrange("p (d s) -> p d s", d=DC)[:, :, 0:SB],
                hidden_t.ap()[:, 0:SB].rearrange("(d p) s -> p d s", d=DC),
            )
            bias = pp.tile([128, 4], f32, tag="bias", name="bias")
            nc.sync.dma_start(bias[:], b_qkv.ap())
            wvt = pp.tile([128, 2 * C], bf16, tag="wvt", name="wvt")
            nc.sync.dma_start(wvt[:], wv_in.ap())
            mask4 = pp.tile([128, 4 * KT], bf16, tag="mask4", name="mask4")
            nc.sync.dma_start(mask4[:], mask_in.ap())
            nc.sync.dma_start(
                hT_all[:].rearrange("p (d s) -> p d s", d=DC)[:, :, SB:S],
                hidden_t.ap()[:, SB:S].rearrange("(d p) s -> p d s", d=DC),
            )
            wpj = pp.tile([128, D], bf16, tag="wpj", name="wpj")
            nc.sync.dma_start(wpj[:], w_projp.ap())

            def wq_sl(d, mb):
                return wq_all[:, d * 512 + mb * 128:d * 512 + (mb + 1) * 128]

            def hT_sl(d, lo, hi):
                return hT_all[:, d * S + lo:d * S + hi]

            ones = pp.tile([128, 1], bf16, tag="ones", name="ones")
            nc.vector.memset(ones[:], 1.0)
            # preload the Exp table set early (off the critical path)
            warm = pp.tile([128, 1], bf16, tag="warm", name="warm")
            nc.scalar.activation(warm[:], ones[:], Exp)

            # qkv'^T destinations
            qp = pp.tile([128, S], bf16, tag="qp", name="qp")
            kcT = pp.tile([128, S], bf16, tag="kcT", name="kcT")
            vT = [pp.tile([128, S], bf16, tag=f"vT{p}", name=f"vT{p}") for p in range(2)]
            dests = [qp, kcT, vT[0], vT[1]]
            vpack = [pp.tile([128, 128], bf16, tag=f"vpk{t}", name=f"vpk{t}")
                     for t in range(NKT)]

            def qkv_group(sb, mb):
                ps = ps_big.tile([128, SB], f32, tag="big", name="psQ")
                for d in range(DC):
                    nc.tensor.matmul(
                        ps[:],
                        wq_sl(d, mb),
                        hT_sl(d, sb * SB, (sb + 1) * SB),
                        start=(d == 0),
                        stop=(d == DC - 1),
                    )
                nc.vector.tensor_scalar_add(
                    out=dests[mb][:, sb * SB:(sb + 1) * SB],
                    in0=ps[:],
                    scalar1=bias[:, mb:mb + 1],
                )

            def vc_pair(kt, p):
                # heads 2p (rows 0-63) and 2p+1 (rows 64-127) of vT[p]
                psa = ps_big.tile([128, SB], f32, tag="big", name="psVa")
                psb = ps_big.tile([128, SB], f32, tag="big", name="psVb")
                nc.tensor.matmul(
                    psa[:, 0:C], vT[p][0:64, kt * KT:(kt + 1) * KT],
                    wvt[0:64, C * p:C * p + C], tile_position=(0, 0),
                )
                nc.tensor.matmul(
                    psb[:, 0:C], vT[p][64:128, kt * KT:(kt + 1) * KT],
                    wvt[64:128, C * p:C * p + C], tile_position=(64, 0),
                )
                nc.vector.tensor_copy(
                    vpack[kt][:, 64 * p:64 * p + C], psa[:, 0:C])
                nc.vector.tensor_copy(
                    vpack[kt][:, 64 * p + C:64 * p + 2 * C], psb[:, 0:C])



            es = {}

            def scores_half(qsb, kb, half):
                # half 0: heads 0,1 -> scA; half 1: heads 2,3 -> scB
                r = kb - 4 * qsb
                c0 = max(r, 0) * KT
                pool = ps_scA if half == 0 else ps_scB
                sc = pool.tile([128, 2 * SB], f32, tag=f"sc{half}", name="sc")
                for hh in range(2):
                    h = 2 * half + hh
                    nc.tensor.matmul(
                        sc[:, hh * SB + c0:(hh + 1) * SB],
                        kcT[32 * h:32 * h + 32, kb * KT:(kb + 1) * KT],
                        qp[32 * h:32 * h + 32, qsb * SB + c0:(qsb + 1) * SB],
                        tile_position=(32 * h, 0),
                    )
                if half == 0:
                    es[(qsb, kb)] = ep.tile([128, 4 * SB], bf16, tag="e", name="e")
                e = es[(qsb, kb)]
                scv = sc[:].rearrange("p (h w) -> p h w", h=2)[:, :, c0:SB]
                ev = e[:].rearrange("p (h w) -> p h w", h=4)[:, 2 * half:2 * half + 2, c0:SB]
                nc.scalar.activation(ev, scv, Exp)
                if r >= 0 and half == 1:
                    # mask the diagonal 128-col block of each head (all 4)
                    ed = e[:].rearrange("p (h w) -> p h w", h=4)[:, :, c0:c0 + KT]
                    mv = mask4[:].rearrange("p (h w) -> p h w", h=4)
                    nc.vector.tensor_tensor(ed, ed, mv, mult)

            def scores_group(qsb, kb):
                scores_half(qsb, kb, 0)
                scores_half(qsb, kb, 1)

            attn_ps = {}
            den_ps = {}

            def attn_group(qsb, kb):
                r = kb - 4 * qsb
                c0 = max(r, 0) * KT
                nkb = 4 * qsb + 4
                if kb == 0:
                    attn_ps[qsb] = ps_at.tile([128, SB], f32, tag="at", name="at")
                    den_ps[qsb] = ps_dn.tile([128, SB], f32, tag="dn", name="dn")
                at, dn = attn_ps[qsb], den_ps[qsb]
                e = es.pop((qsb, kb))
                for h in range(HPC):
                    ee = e[:, h * SB + c0:(h + 1) * SB]
                    nc.tensor.matmul(
                        at[32 * h:32 * h + 32, c0:SB], vpack[kb][:, 32 * h:32 * h + 32],
                        ee, tile_position=(0, 32 * h),
                        start=(kb == 0), stop=(kb == nkb - 1),
                    )
                    nc.tensor.matmul(
                        dn[32 * h:32 * h + 1, c0:SB], ones[:, 0:1],
                        ee, tile_position=(0, 32 * h),
                        start=(kb == 0), stop=(kb == nkb - 1),
                    )

            attn_norm = {}

            def norm(qsb):
                at = attn_ps.pop(qsb)
                dn = den_ps.pop(qsb)
                attn_sb = npo.tile([128, SB], bf16, tag="attn_sb", name="attn_sb")
                nc.vector.tensor_copy(attn_sb[:], at[:])
                den_sb = npo.tile([128, SB], bf16, tag="den_sb", name="den_sb")
                nc.vector.tensor_copy(den_sb[:], dn[:])
                # bounce den rows {32h} -> [128,16] for a cheap DVE reciprocal
                den_dr = dr.tile([HPC, SB], bf16, tag="den_dr", name="den_dr")
                nc.sync.dma_start(
                    den_dr[:], den_sb[:].rearrange("(h s) q -> h s q", s=32)[:, 0, :]
                )
                den_c = npo.tile([128, 16], bf16, tag="den_c", name="den_c")
                nc.sync.dma_start(
                    den_c[:], den_dr[:].rearrange("h (p j) -> (h p) j", j=16)
                )
                rec_c = npo.tile([128, 16], bf16, tag="rec_c", name="rec_c")
                with nc.allow_low_precision(reason="softmax denom recip in bf16"):
                    nc.vector.reciprocal(rec_c[:], den_c[:])
                rec_dr = dr.tile([HPC, SB], bf16, tag="rec_dr", name="rec_dr")
                nc.sync.dma_start(
                    rec_dr[:].rearrange("h (p j) -> (h p) j", j=16), rec_c[:]
                )
                recb = npo.tile([128, SB], bf16, tag="recb", name="recb")
                for h in range(HPC):
                    nc.sync.dma_start(
                        recb[32 * h:32 * h + 32, :],
                        rec_dr[h:h + 1, :].to_broadcast([32, SB]),
                    )
                an = npo.tile([128, SB], bf16, tag="an", name="an")
                nc.vector.tensor_tensor(an[:], attn_sb[:], recb[:], mult)
                attn_norm[qsb] = an

            def proj_group(qsb, mb):
                ps = ps_big.tile([128, SB], f32, tag="big", name="psP")
                nc.tensor.matmul(
                    ps[:], wpj[:, mb * 128:(mb + 1) * 128], attn_norm[qsb][:],
                )
                stage = op.tile([128, SB], bf16, tag="stage", name="stage")
                nc.vector.tensor_copy(stage[:], ps[:])
                nc.sync.dma_start(
                    out_t.ap()[mb * 128:(mb + 1) * 128, qsb * SB:(qsb + 1) * SB],
                    stage[:],
                )

            # ================= PE schedule =================
            # Filler work (qkv of later s-blocks, v_c compress, c_proj of
            # earlier q-blocks) is spread just-in-time across the qsb windows
            # so the PE stays dense (HAM-warm) while pacing the ScalarE exps.
            def QKV(sb, mb):
                return lambda: qkv_group(sb, mb)

            def VC(kt):
                def f():
                    vc_pair(kt, 0)
                    vc_pair(kt, 1)
                return f

            def PRJ(qsb, mb):
                return lambda: proj_group(qsb, mb)

            fillers = {
                0: [[QKV(1, 0), VC(2)], [QKV(1, 1), VC(3)],
                    [QKV(1, 2)], [QKV(1, 3)]],
                1: [[QKV(2, 0)], [QKV(2, 1)], [QKV(2, 2)],
                    [QKV(2, 3), VC(4)], [VC(5)], [VC(6)], [VC(7)], []],
                2: [[QKV(3, 0)], [QKV(3, 1)], [QKV(3, 2)], [QKV(3, 3)],
                    [PRJ(0, 0)], [PRJ(0, 1), VC(8)], [PRJ(0, 2), VC(9)],
                    [PRJ(0, 3), VC(10)], [PRJ(0, 4), VC(11)],
                    [PRJ(0, 5)], [PRJ(0, 6)], [PRJ(0, 7)]],
                3: [[PRJ(1, 0)], [PRJ(1, 1)], [PRJ(1, 2)], [PRJ(1, 3)],
                    [PRJ(1, 4)], [PRJ(1, 5)], [PRJ(1, 6)], [PRJ(1, 7)],
                    [PRJ(2, 0), VC(12)], [PRJ(2, 1), VC(13)],
                    [PRJ(2, 2), VC(14)], [PRJ(2, 3), VC(15)],
                    [PRJ(2, 4)], [PRJ(2, 5)], [PRJ(2, 6)], [PRJ(2, 7)]],
            }

            def run_filler(slot):
                for thunk in slot:
                    thunk()

            # lead-in: q'/k_c of sb0, first scores ASAP, then v of sb0
            qkv_group(0, 0)
            qkv_group(0, 1)
            scores_group(0, 0)
            qkv_group(0, 2)
            qkv_group(0, 3)
            vc_pair(0, 0)
            vc_pair(0, 1)
            vc_pair(1, 0)
            vc_pair(1, 1)

            for qsb in range(NSB):
                nkb = 4 * qsb + 4
                fl = fillers[qsb]
                for kb in range(nkb):
                    if not (qsb == 0 and kb == 0):
                        scores_group(qsb, kb)
                    if kb >= 1:
                        attn_group(qsb, kb - 1)
                    if kb < len(fl):
                        run_filler(fl[kb])
                attn_group(qsb, nkb - 1)
                norm(qsb)
            for mb in range(8):
                proj_group(3, mb)

    nc.compile()
    return nc


def _prep_inputs(hidden_states, w_attn, b_attn, wk_c, wv_c, wk_d, wv_d, w_proj):
    """Per-core input maps (host-side shard + fold + pack + bf16 cast)."""
    f8 = np.float64
    hidden_T = [np.ascontiguousarray(hidden_states[b].T).astype(BF16) for b in range(B)]
    w_attn8, b_attn8 = w_attn.astype(f8), b_attn.astype(f8)
    scl = 1.0 / np.sqrt(hd)

    k = np.arange(128).reshape(128, 1)
    j = np.arange(KT).reshape(1, KT)
    mask1 = (k <= j).astype(BF16)
    mask4 = np.tile(mask1, (1, 4))

    in_maps = []
    for c in range(NCORES):
        b = c // 4
        hs = [4 * (c % 4) + h for h in range(HPC)]
        wq_cols, bq = [], []
        wk_cols, bk = [], []
        for h in hs:
            wq_cols.append(w_attn8[:, h * hd:(h + 1) * hd] @ wk_d[h].astype(f8).T * scl)
            bq.append(b_attn8[h * hd:(h + 1) * hd] @ wk_d[h].astype(f8).T * scl)
            wk_cols.append(w_attn8[:, D + h * hd:D + (h + 1) * hd] @ wk_c[h].astype(f8))
            bk.append(b_attn8[D + h * hd:D + (h + 1) * hd] @ wk_c[h].astype(f8))
        wv_cols = [w_attn[:, 2 * D + h * hd:2 * D + (h + 1) * hd] for h in hs]
        bv = [b_attn[2 * D + h * hd:2 * D + (h + 1) * hd] for h in hs]
        w_qkv_l = np.concatenate(
            [np.concatenate(wq_cols, 1), np.concatenate(wk_cols, 1),
             np.concatenate(wv_cols[0:2], 1), np.concatenate(wv_cols[2:4], 1)],
            axis=1,
        ).astype(BF16)                                          # [1024, 512]
        b_qkv_l = np.stack(
            [np.concatenate(bq), np.concatenate(bk),
             np.concatenate(bv[0:2]), np.concatenate(bv[2:4])],
            axis=1,
        ).astype(np.float32)                                    # [128, 4]
        wv_l = np.zeros((128, 2 * C), BF16)
        for p in range(2):
            wv_l[0:64, C * p:C * p + C] = wv_c[hs[2 * p]].astype(BF16)
            wv_l[64:128, C * p:C * p + C] = wv_c[hs[2 * p + 1]].astype(BF16)
        w_projp_l = np.concatenate(
            [wv_d[h].astype(f8) @ w_proj[h * hd:(h + 1) * hd, :].astype(f8)
             for h in hs], axis=0,
        ).astype(BF16)                                          # [128, 1024]
        in_maps.append(
            {
                "hidden_t": hidden_T[b],
                "w_qkv": w_qkv_l,
                "b_qkv": b_qkv_l,
                "wv_in": wv_l,
                "w_projp": w_projp_l,
                "mask_in": mask4,
            }
        )
    return in_maps


def kernel(
    hidden_states,
    w_attn,
    b_attn,
    w_proj,
    b_proj,
    wk_c,
    wv_c,
    wk_d,
    wv_d,
    _trace=False,
):
    from concourse.bass_utils import run_bass_kernel_spmd

    if "nc" not in _cache:
        _cache["nc"] = _build()
    nc = _cache["nc"]

    in_maps = _prep_inputs(
        np.asarray(hidden_states),
        np.asarray(w_attn),
        np.asarray(b_attn),
        np.asarray(wk_c),
        np.asarray(wv_c),
        np.asarray(wk_d),
        np.asarray(wv_d),
        np.asarray(w_proj),
    )
    res = run_bass_kernel_spmd(
        nc, in_maps, core_ids=list(range(NCORES)), trace=_trace
    )
    out = np.empty((B, S, D), np.float32)
    for b in range(B):
        acc = np.zeros((D, S), np.float32)
        for c in range(4 * b, 4 * b + 4):
            acc += res.results[c]["out_t"].astype(np.float32)
        out[b] = acc.T + np.asarray(b_proj, np.float32)
    if _trace:
        _cache["last_exec_time_ns"] = res.exec_time_ns
        _cache["last_results"] = res
    return out


# revision 16
# speedup vs baseline: 1.8283x; 1.1010x over previous
"""Compressed-KV GPT-2 attention block on 8 TRN2 NeuronCores.

Sharding: batch x head-group. Core c: batch b = c//4, heads 4*(c%4)..+4.

The KV compressor is linear + low-rank, so everything folds on host:
  scores = q @ (k_c wk_d)^T / 8 = (q wk_d^T / 8) @ k_c^T   -> q' [S,32]
  v_c    = v @ wv_c = hidden @ (w_v wv_c) + b_v wv_c       -> direct projection
  out_h  = (P @ v_c) @ (wv_d w_proj_h)                     -> attn in C=32 space
so the device pipeline works entirely in the compressed C=32 head space:
  qkv'  : hidden^T -> q'^T, k_c^T, v_c^T (32 rows/head, 3 m-blocks)
  vpack : per key tile, PE-transpose v_c^T [128,128] -> v_c [keys, 4hx32c]
  S^T   : k_c^T slices^T @ q'^T, 2+2 heads packed via 32x128-mode PE row
          tiling into two 2-bank PSUM tiles (scA/scB); exp on ScalarE per half
          ([128,~1024] strided) with zero ping-pong bubbles
  attn  : v_c^T @ E + ones^T @ E (denominator), 4 heads packed via 128x32-mode
          PE column tiling; normalize by DMA-bounced reciprocal broadcast
  out^T : w_proj'^T-chunks @ attn_norm (K=128 covers all 4 heads at C=32)
Host sums the 4 partials per batch and adds b_proj.

The PE instruction stream hand-interleaves qkv/vpack/proj filler work into the
score/attnV stream so the PE stays dense (HAM-warm) while pacing the ScalarE
exp stream (the bottleneck). The last q-block's normalize + c_proj are split
into column halves so most of the tail overlaps the final exps.
"""

import sys

if "/opt/trn_rl_repo" not in sys.path:
    sys.path.insert(0, "/opt/trn_rl_repo")

import numpy as np
import ml_dtypes

BF16 = ml_dtypes.bfloat16

B, S, D = 2, 2048, 1024
H, hd, C = 16, 64, 32
NCORES = 8
HPC = 4            # heads per core
SB = 512           # q block
NSB = S // SB      # 4
KT = 128           # keys per tile
NKT = S // KT      # 16
DC = D // 128      # 8 contraction chunks
MB = 3             # qkv m-blocks: q', k_c, v_c

_cache = {}


def _build():
    import concourse.bacc as bacc
    import concourse.tile as tile
    import concourse.mybir as mybir

    dt = mybir.dt
    f32, bf16 = dt.float32, dt.bfloat16
    Exp = mybir.ActivationFunctionType.Exp
    mult = mybir.AluOpType.mult

    nc = bacc.Bacc("TRN2", target_bir_lowering=False, debug=False, num_devices=NCORES)

    hidden_t = nc.dram_tensor("hidden_t", [D, S], bf16, kind="ExternalInput")
    w_qkv = nc.dram_tensor("w_qkv", [D, MB * 128], bf16, kind="ExternalInput")
    b_qkv = nc.dram_tensor("b_qkv", [128, MB], f32, kind="ExternalInput")
    w_projp = nc.dram_tensor("w_projp", [128, D], bf16, kind="ExternalInput")
    mask_in = nc.dram_tensor("mask_in", [128, 4 * KT], bf16, kind="ExternalInput")
    ident_in = nc.dram_tensor("ident_in", [128, 128], bf16, kind="ExternalInput")
    out_t = nc.dram_tensor("out_t", [D, S], bf16, kind="ExternalOutput")

    with tile.TileContext(nc) as tc:
        with (
            tc.tile_pool(name="persist", bufs=1) as pp,
            tc.tile_pool(name="epool", bufs=6) as ep,
            tc.tile_pool(name="npool", bufs=2) as npo,
            tc.tile_pool(name="ostage", bufs=3) as op,
            tc.tile_pool(name="dscr", bufs=2, space="DRAM") as dr,
            tc.tile_pool(name="ps_scA", bufs=1, space="PSUM") as ps_scA,
            tc.tile_pool(name="ps_scB", bufs=1, space="PSUM") as ps_scB,
            tc.tile_pool(name="ps_at", bufs=1, space="PSUM") as ps_at,
            tc.tile_pool(name="ps_dn", bufs=1, space="PSUM") as ps_dn,
            tc.tile_pool(name="ps_big", bufs=2, space="PSUM") as ps_big,
        ):
            # ---- weights + hidden, few big DMAs, in consumption order ----
            wq_all = pp.tile([128, DC * MB * 128], bf16, tag="wq", name="wq_all")
            nc.sync.dma_start(
                wq_all[:].rearrange("p (d c) -> p d c", d=DC)[:, :, 0:256],
                w_qkv.ap()[:, 0:256].rearrange("(d p) c -> p d c", d=DC),
            )
            hT_all = pp.tile([128, DC * S], bf16, tag="hT", name="hT_all")
            hv = hT_all[:].rearrange("p (d s) -> p d s", d=DC)
            nc.sync.dma_start(
                hv[:, 0:4, 0:SB],
                hidden_t.ap()[0:512, 0:SB].rearrange("(d p) s -> p d s", d=4),
            )
            nc.sync.dma_start(
                hv[:, 4:8, 0:SB],
                hidden_t.ap()[512:1024, 0:SB].rearrange("(d p) s -> p d s", d=4),
            )
            bias = pp.tile([128, MB], f32, tag="bias", name="bias")
            nc.sync.dma_start(bias[:], b_qkv.ap())
            nc.sync.dma_start(
                wq_all[:].rearrange("p (d c) -> p d c", d=DC)[:, :, 256:384],
                w_qkv.ap()[:, 256:384].rearrange("(d p) c -> p d c", d=DC),
            )
            ident = pp.tile([128, 128], bf16, tag="ident", name="ident")
            nc.sync.dma_start(ident[:], ident_in.ap())
            mask4 = pp.tile([128, 4 * KT], bf16, tag="mask4", name="mask4")
            nc.sync.dma_start(mask4[:], mask_in.ap())
            nc.sync.dma_start(
                hv[:, :, SB:S],
                hidden_t.ap()[:, SB:S].rearrange("(d p) s -> p d s", d=DC),
            )
            wpj = pp.tile([128, D], bf16, tag="wpj", name="wpj")
            nc.sync.dma_start(wpj[:], w_projp.ap())

            def wq_sl(d, mb):
                return wq_all[:, d * (MB * 128) + mb * 128:d * (MB * 128) + (mb + 1) * 128]

            def hT_sl(d, lo, hi):
                return hT_all[:, d * S + lo:d * S + hi]

            ones = pp.tile([128, 1], bf16, tag="ones", name="ones")
            nc.vector.memset(ones[:], 1.0)
            # preload the Exp table set early (off the critical path)
            warm = pp.tile([128, 1], bf16, tag="warm", name="warm")
            nc.scalar.activation(warm[:], ones[:], Exp)

            # qkv'^T destinations
            qp = pp.tile([128, S], bf16, tag="qp", name="qp")
            kcT = pp.tile([128, S], bf16, tag="kcT", name="kcT")
            vcT = pp.tile([128, S], bf16, tag="vcT", name="vcT")
            dests = [qp, kcT, vcT]
            vpack = [pp.tile([128, 128], bf16, tag=f"vpk{t}", name=f"vpk{t}")
                     for t in range(NKT)]

            def qkv_group(sb, mb):
                ps = ps_big.tile([128, SB], f32, tag="big", name="psQ")
                for d in range(DC):
                    nc.tensor.matmul(
                        ps[:],
                        wq_sl(d, mb),
                        hT_sl(d, sb * SB, (sb + 1) * SB),
                        start=(d == 0),
                        stop=(d == DC - 1),
                    )
                nc.vector.tensor_scalar_add(
                    out=dests[mb][:, sb * SB:(sb + 1) * SB],
                    in0=ps[:],
                    scalar1=bias[:, mb:mb + 1],
                )

            def vc_tr(kt):
                # vpack[kt][key, 32h+c] = vcT[32h+c, kt*128+key] via PE transpose
                psT = ps_big.tile([128, 128], bf16, tag="big", name="psT")
                nc.tensor.transpose(
                    psT[:], vcT[:, kt * KT:(kt + 1) * KT], ident[:]
                )
                nc.vector.tensor_copy(vpack[kt][:], psT[:])

            es = {}

            def scores_half(qsb, kb, half):
                # half 0: heads 0,1 -> scA; half 1: heads 2,3 -> scB
                r = kb - 4 * qsb
                c0 = max(r, 0) * KT
                pool = ps_scA if half == 0 else ps_scB
                sc = pool.tile([128, 2 * SB], f32, tag=f"sc{half}", name="sc")
                for hh in range(2):
                    h = 2 * half + hh
                    nc.tensor.matmul(
                        sc[:, hh * SB + c0:(hh + 1) * SB],
                        kcT[32 * h:32 * h + 32, kb * KT:(kb + 1) * KT],
                        qp[32 * h:32 * h + 32, qsb * SB + c0:(qsb + 1) * SB],
                        tile_position=(32 * h, 0),
                    )
                if half == 0:
                    es[(qsb, kb)] = ep.tile([128, 4 * SB], bf16, tag="e", name="e")
                e = es[(qsb, kb)]
                scv = sc[:].rearrange("p (h w) -> p h w", h=2)[:, :, c0:SB]
                ev = e[:].rearrange("p (h w) -> p h w", h=4)[:, 2 * half:2 * half + 2, c0:SB]
                nc.scalar.activation(ev, scv, Exp)
                if r >= 0 and half == 1:
                    # mask the diagonal 128-col block of each head (all 4)
                    ed = e[:].rearrange("p (h w) -> p h w", h=4)[:, :, c0:c0 + KT]
                    mv = mask4[:].rearrange("p (h w) -> p h w", h=4)
                    nc.vector.tensor_tensor(ed, ed, mv, mult)

            def scores_group(qsb, kb):
                scores_half(qsb, kb, 0)
                scores_half(qsb, kb, 1)

            attn_ps = {}
            den_ps = {}

            def attn_group(qsb, kb):
                r = kb - 4 * qsb
                c0 = max(r, 0) * KT
                nkb = 4 * qsb + 4
                if kb == 0:
                    attn_ps[qsb] = ps_at.tile([128, SB], f32, tag="at", name="at")
                    den_ps[qsb] = ps_dn.tile([128, SB], f32, tag="dn", name="dn")
                at, dn = attn_ps[qsb], den_ps[qsb]
                e = es.pop((qsb, kb))
                for h in range(HPC):
                    ee = e[:, h * SB + c0:(h + 1) * SB]
                    nc.tensor.matmul(
                        at[32 * h:32 * h + 32, c0:SB], vpack[kb][:, 32 * h:32 * h + 32],
                        ee, tile_position=(0, 32 * h),
                        start=(kb == 0), stop=(kb == nkb - 1),
                    )
                    nc.tensor.matmul(
                        dn[32 * h:32 * h + 1, c0:SB], ones[:, 0:1],
                        ee, tile_position=(0, 32 * h),
                        start=(kb == 0), stop=(kb == nkb - 1),
                    )

            attn_norm = {}

            def norm(qsb, lo, hi, key):
                # normalize attn columns [lo, hi) of q-block qsb
                w = hi - lo
                at, dn = attn_ps[qsb], den_ps[qsb]
                attn_sb = npo.tile([128, w], bf16, tag=f"attn_sb{w}", name="attn_sb")
                nc.vector.tensor_copy(attn_sb[:], at[:, lo:hi])
                den_sb = npo.tile([128, w], bf16, tag=f"den_sb{w}", name="den_sb")
                nc.vector.tensor_copy(den_sb[:], dn[:, lo:hi])
                # bounce den rows {32h} -> [128, w/32] for a cheap DVE reciprocal
                den_dr = dr.tile([HPC, w], bf16, tag=f"den_dr{w}", name="den_dr")
                nc.sync.dma_start(
                    den_dr[:], den_sb[:].rearrange("(h s) q -> h s q", s=32)[:, 0, :]
                )
                j = w // 32
                den_c = npo.tile([128, j], bf16, tag=f"den_c{w}", name="den_c")
                nc.sync.dma_start(
                    den_c[:], den_dr[:].rearrange("h (p j) -> (h p) j", j=j)
                )
                rec_c = npo.tile([128, j], bf16, tag=f"rec_c{w}", name="rec_c")
                with nc.allow_low_precision(reason="softmax denom recip in bf16"):
                    nc.vector.reciprocal(rec_c[:], den_c[:])
                rec_dr = dr.tile([HPC, w], bf16, tag=f"rec_dr{w}", name="rec_dr")
                nc.sync.dma_start(
                    rec_dr[:].rearrange("h (p j) -> (h p) j", j=j), rec_c[:]
                )
                recb = npo.tile([128, w], bf16, tag=f"recb{w}", name="recb")
                for h in range(HPC):
                    nc.sync.dma_start(
                        recb[32 * h:32 * h + 32, :],
                        rec_dr[h:h + 1, :].to_broadcast([32, w]),
                    )
                an = npo.tile([128, w], bf16, tag=f"an{w}", name="an")
                nc.vector.tensor_tensor(an[:], attn_sb[:], recb[:], mult)
                attn_norm[key] = (an, lo, hi)

            def proj_group(key, mb):
                an, lo, hi = attn_norm[key]
                qsb = key if isinstance(key, int) else key[0]
                w = hi - lo
                ps = ps_big.tile([128, SB], f32, tag="big", name="psP")
                nc.tensor.matmul(ps[:, 0:w], wpj[:, mb * 128:(mb + 1) * 128], an[:])
                stage = op.tile([128, w], bf16, tag=f"stage{w}", name="stage")
                nc.vector.tensor_copy(stage[:], ps[:, 0:w])
                nc.sync.dma_start(
                    out_t.ap()[mb * 128:(mb + 1) * 128,
                               qsb * SB + lo:qsb * SB + hi],
                    stage[:],
                )

            # ================= PE schedule =================
            def QKV(sb, mb):
                return lambda: qkv_group(sb, mb)

            def VCT(kt):
                return lambda: vc_tr(kt)

            def PRJ(key, mb):
                return lambda: proj_group(key, mb)

            def NRM3A():
                def f():
                    norm(3, 0, 256, (3, 0))
                    for mb in range(8):
                        proj_group((3, 0), mb)
                return f

            fillers = {
                0: [[QKV(1, 0), VCT(2)], [QKV(1, 1), VCT(3)],
                    [QKV(1, 2)], []],
                1: [[QKV(2, 0)], [QKV(2, 1)], [QKV(2, 2)],
                    [VCT(4)], [VCT(5)], [VCT(6)], [VCT(7)], []],
                2: [[QKV(3, 0)], [QKV(3, 1)], [QKV(3, 2)],
                    [PRJ(0, 0)], [PRJ(0, 1)], [PRJ(0, 2), VCT(8)],
                    [PRJ(0, 3), VCT(9)], [PRJ(0, 4), VCT(10)],
                    [PRJ(0, 5), VCT(11)], [PRJ(0, 6)], [PRJ(0, 7)], []],
                3: [[PRJ(1, 0)], [PRJ(1, 1)], [PRJ(1, 2)], [PRJ(1, 3)],
                    [PRJ(1, 4)], [PRJ(1, 5)], [PRJ(1, 6)], [PRJ(1, 7)],
                    [PRJ(2, 0), VCT(12)], [PRJ(2, 1), VCT(13)],
                    [PRJ(2, 2), VCT(14)], [PRJ(2, 3), VCT(15)],
                    [PRJ(2, 4)], [PRJ(2, 5)], [PRJ(2, 6)],
                    [PRJ(2, 7), NRM3A()]],
            }

            # lead-in: q'/k_c of sb0, first scores ASAP, then v_c of sb0
            qkv_group(0, 0)
            qkv_group(0, 1)
            scores_group(0, 0)
            qkv_group(0, 2)
            vc_tr(0)
            vc_tr(1)

            for qsb in range(NSB):
                nkb = 4 * qsb + 4
                fl = fillers[qsb]
                for kb in range(nkb):
                    if not (qsb == 0 and kb == 0):
                        scores_group(qsb, kb)
                    if kb >= 1:
                        attn_group(qsb, kb - 1)
                    if kb < len(fl):
                        for thunk in fl[kb]:
                            thunk()
                attn_group(qsb, nkb - 1)
                if qsb < 3:
                    norm(qsb, 0, SB, qsb)
                    attn_ps.pop(qsb)
                    den_ps.pop(qsb)
            # tail: second half of qsb3
            norm(3, 256, SB, (3, 1))
            attn_ps.pop(3)
            den_ps.pop(3)
            for mb in range(8):
                proj_group((3, 1), mb)

    nc.compile()
    return nc


def _prep_inputs(hidden_states, w_attn, b_attn, wk_c, wv_c, wk_d, wv_d, w_proj):
    """Per-core input maps (host-side shard + fold + pack + bf16 cast)."""
    f8 = np.float64
    hidden_T = [np.ascontiguousarray(hidden_states[b].T).astype(BF16) for b in range(B)]
    w_attn8, b_attn8 = w_attn.astype(f8), b_attn.astype(f8)
    scl = 1.0 / np.sqrt(hd)

    k = np.arange(128).reshape(128, 1)
    j = np.arange(KT).reshape(1, KT)
    mask1 = (k <= j).astype(BF16)
    mask4 = np.tile(mask1, (1, 4))
    ident = np.eye(128, dtype=BF16)

    in_maps = []
    for c in range(NCORES):
        b = c // 4
        hs = [4 * (c % 4) + h for h in range(HPC)]
        wq_cols, bq = [], []
        wk_cols, bk = [], []
        wv_cols, bv = [], []
        for h in hs:
            wq_cols.append(w_attn8[:, h * hd:(h + 1) * hd] @ wk_d[h].astype(f8).T * scl)
            bq.append(b_attn8[h * hd:(h + 1) * hd] @ wk_d[h].astype(f8).T * scl)
            wk_cols.append(w_attn8[:, D + h * hd:D + (h + 1) * hd] @ wk_c[h].astype(f8))
            bk.append(b_attn8[D + h * hd:D + (h + 1) * hd] @ wk_c[h].astype(f8))
            wv_cols.append(
                w_attn8[:, 2 * D + h * hd:2 * D + (h + 1) * hd] @ wv_c[h].astype(f8))
            bv.append(b_attn8[2 * D + h * hd:2 * D + (h + 1) * hd] @ wv_c[h].astype(f8))
        w_qkv_l = np.concatenate(
            [np.concatenate(wq_cols, 1), np.concatenate(wk_cols, 1),
             np.concatenate(wv_cols, 1)], axis=1,
        ).astype(BF16)                                          # [1024, 384]
        b_qkv_l = np.stack(
            [np.concatenate(bq), np.concatenate(bk), np.concatenate(bv)],
            axis=1,
        ).astype(np.float32)                                    # [128, 3]
        w_projp_l = np.concatenate(
            [wv_d[h].astype(f8) @ w_proj[h * hd:(h + 1) * hd, :].astype(f8)
             for h in hs], axis=0,
        ).astype(BF16)                                          # [128, 1024]
        in_maps.append(
            {
                "hidden_t": hidden_T[b],
                "w_qkv": w_qkv_l,
                "b_qkv": b_qkv_l,
                "w_projp": w_projp_l,
                "mask_in": mask4,
                "ident_in": ident,
            }
        )
    return in_maps


def kernel(
    hidden_states,
    w_attn,
    b_attn,
    w_proj,
    b_proj,
    wk_c,
    wv_c,
    wk_d,
    wv_d,
    _trace=False,
):
    from concourse.bass_utils import run_bass_kernel_spmd

    if "nc" not in _cache:
        _cache["nc"] = _build()
    nc = _cache["nc"]

    in_maps = _prep_inputs(
        np.asarray(hidden_states),
        np.asarray(w_attn),
        np.asarray(b_attn),
        np.asarray(wk_c),
        np.asarray(wv_c),
        np.asarray(wk_d),
        np.asarray(wv_d),
        np.asarray(w_proj),
    )
    res = run_bass_kernel_spmd(
        nc, in_maps, core_ids=list(range(NCORES)), trace=_trace
    )
    out = np.empty((B, S, D), np.float32)
    for b in range(B):
        acc = np.zeros((D, S), np.float32)
        for c in range(4 * b, 4 * b + 4):
            acc += res.results[c]["out_t"].astype(np.float32)
        out[b] = acc.T + np.asarray(b_proj, np.float32)
    if _trace:
        _cache["last_exec_time_ns"] = res.exec_time_ns
        _cache["last_results"] = res
    return out


# revision 23
# speedup vs baseline: 1.8292x; 1.0005x over previous
"""Compressed-KV GPT-2 attention block on 8 TRN2 NeuronCores.

Sharding: batch x head-group. Core c: batch b = c//4, heads 4*(c%4)..+4.

The KV compressor is linear + low-rank, so everything folds on host:
  scores = q @ (k_c wk_d)^T / 8 = (q wk_d^T / 8) @ k_c^T   -> q' [S,32]
  v_c    = v @ wv_c = hidden @ (w_v wv_c) + b_v wv_c       -> direct projection
  out_h  = (P @ v_c) @ (wv_d w_proj_h)                     -> attn in C=32 space
so the device pipeline works entirely in the compressed C=32 head space:
  qkv'  : hidden^T -> q'^T, k_c^T, v_c^T (32 rows/head, 3 m-blocks)
  vpack : per key tile, PE-transpose v_c^T [128,128] -> v_c [keys, 4hx32c]
  S^T   : k_c^T slices^T @ q'^T, 2+2 heads packed via 32x128-mode PE row
          tiling into two 2-bank PSUM tiles (scA/scB); exp on ScalarE per half
          ([128,~1024] strided) with zero ping-pong bubbles
  attn  : v_c^T @ E + ones^T @ E (denominator), 4 heads packed via 128x32-mode
          PE column tiling; normalize by DMA-bounced reciprocal broadcast
  out^T : w_proj'^T-chunks @ attn_norm (K=128 covers all 4 heads at C=32)
Host sums the 4 partials per batch and adds b_proj.

The PE instruction stream hand-interleaves qkv/vpack/proj filler work into the
score/attnV stream so the PE stays dense (HAM-warm) while pacing the ScalarE
exp stream (the bottleneck). The last q-block's normalize + c_proj are split
into column halves so most of the tail overlaps the final exps.
"""

import sys

if "/opt/trn_rl_repo" not in sys.path:
    sys.path.insert(0, "/opt/trn_rl_repo")

import numpy as np
import ml_dtypes

BF16 = ml_dtypes.bfloat16

B, S, D = 2, 2048, 1024
H, hd, C = 16, 64, 32
NCORES = 8
HPC = 4            # heads per core
SB = 512           # q block
NSB = S // SB      # 4
KT = 128           # keys per tile
NKT = S // KT      # 16
DC = D // 128      # 8 contraction chunks
MB = 3             # qkv m-blocks: q', k_c, v_c

_cache = {}


def _build():
    import concourse.bacc as bacc
    import concourse.tile as tile
    import concourse.mybir as mybir

    dt = mybir.dt
    f32, bf16 = dt.float32, dt.bfloat16
    Exp = mybir.ActivationFunctionType.Exp
    mult = mybir.AluOpType.mult

    nc = bacc.Bacc("TRN2", target_bir_lowering=False, debug=False, num_devices=NCORES)

    hidden_t = nc.dram_tensor("hidden_t", [D, S], bf16, kind="ExternalInput")
    w_qkv = nc.dram_tensor("w_qkv", [D, MB * 128], bf16, kind="ExternalInput")
    b_qkv = nc.dram_tensor("b_qkv", [128, MB], f32, kind="ExternalInput")
    w_projp = nc.dram_tensor("w_projp", [128, D], bf16, kind="ExternalInput")
    mask_in = nc.dram_tensor("mask_in", [128, 4 * KT], bf16, kind="ExternalInput")
    ident_in = nc.dram_tensor("ident_in", [128, 128], bf16, kind="ExternalInput")
    out_t = nc.dram_tensor("out_t", [D, S], bf16, kind="ExternalOutput")

    with tile.TileContext(nc) as tc:
        with (
            tc.tile_pool(name="persist", bufs=1) as pp,
            tc.tile_pool(name="epool", bufs=6) as ep,
            tc.tile_pool(name="npool", bufs=2) as npo,
            tc.tile_pool(name="ostage", bufs=3) as op,
            tc.tile_pool(name="dscr", bufs=2, space="DRAM") as dr,
            tc.tile_pool(name="ps_scA", bufs=1, space="PSUM") as ps_scA,
            tc.tile_pool(name="ps_scB", bufs=1, space="PSUM") as ps_scB,
            tc.tile_pool(name="ps_at", bufs=1, space="PSUM") as ps_at,
            tc.tile_pool(name="ps_dn", bufs=1, space="PSUM") as ps_dn,
            tc.tile_pool(name="ps_big", bufs=2, space="PSUM") as ps_big,
        ):
            # ---- weights + hidden, few big DMAs, in consumption order ----
            wq_all = pp.tile([128, DC * MB * 128], bf16, tag="wq", name="wq_all")
            nc.sync.dma_start(
                wq_all[:].rearrange("p (d c) -> p d c", d=DC)[:, :, 0:256],
                w_qkv.ap()[:, 0:256].rearrange("(d p) c -> p d c", d=DC),
            )
            hT_all = pp.tile([128, DC * S], bf16, tag="hT", name="hT_all")
            hv = hT_all[:].rearrange("p (d s) -> p d s", d=DC)
            for q in range(4):
                nc.sync.dma_start(
                    hv[:, 2 * q:2 * q + 2, 0:SB],
                    hidden_t.ap()[256 * q:256 * q + 256, 0:SB]
                    .rearrange("(d p) s -> p d s", d=2),
                )
            bias = pp.tile([128, MB], f32, tag="bias", name="bias")
            nc.sync.dma_start(bias[:], b_qkv.ap())
            nc.sync.dma_start(
                wq_all[:].rearrange("p (d c) -> p d c", d=DC)[:, :, 256:384],
                w_qkv.ap()[:, 256:384].rearrange("(d p) c -> p d c", d=DC),
            )
            ident = pp.tile([128, 128], bf16, tag="ident", name="ident")
            nc.sync.dma_start(ident[:], ident_in.ap())
            mask4 = pp.tile([128, 4 * KT], bf16, tag="mask4", name="mask4")
            nc.sync.dma_start(mask4[:], mask_in.ap())
            nc.sync.dma_start(
                hv[:, :, SB:S],
                hidden_t.ap()[:, SB:S].rearrange("(d p) s -> p d s", d=DC),
            )
            wpj = pp.tile([128, D], bf16, tag="wpj", name="wpj")
            nc.sync.dma_start(wpj[:], w_projp.ap())

            def wq_sl(d, mb):
                return wq_all[:, d * (MB * 128) + mb * 128:d * (MB * 128) + (mb + 1) * 128]

            def hT_sl(d, lo, hi):
                return hT_all[:, d * S + lo:d * S + hi]

            ones = pp.tile([128, 1], bf16, tag="ones", name="ones")
            nc.vector.memset(ones[:], 1.0)
            # preload the Exp table set early (off the critical path)
            warm = pp.tile([128, 1], bf16, tag="warm", name="warm")
            nc.scalar.activation(warm[:], ones[:], Exp)

            # qkv'^T destinations
            qp = pp.tile([128, S], bf16, tag="qp", name="qp")
            kcT = pp.tile([128, S], bf16, tag="kcT", name="kcT")
            vcT = pp.tile([128, S], bf16, tag="vcT", name="vcT")
            dests = [qp, kcT, vcT]
            vpack = [pp.tile([128, 128], bf16, tag=f"vpk{t}", name=f"vpk{t}")
                     for t in range(NKT)]

            def qkv_group(sb, mb):
                ps = ps_big.tile([128, SB], f32, tag="big", name="psQ")
                for d in range(DC):
                    nc.tensor.matmul(
                        ps[:],
                        wq_sl(d, mb),
                        hT_sl(d, sb * SB, (sb + 1) * SB),
                        start=(d == 0),
                        stop=(d == DC - 1),
                    )
                nc.vector.tensor_scalar_add(
                    out=dests[mb][:, sb * SB:(sb + 1) * SB],
                    in0=ps[:],
                    scalar1=bias[:, mb:mb + 1],
                )

            def vc_tr(kt):
                # vpack[kt][key, 32h+c] = vcT[32h+c, kt*128+key] via PE transpose
                psT = ps_big.tile([128, 128], bf16, tag="big", name="psT")
                nc.tensor.transpose(
                    psT[:], vcT[:, kt * KT:(kt + 1) * KT], ident[:]
                )
                nc.vector.tensor_copy(vpack[kt][:], psT[:])

            es = {}

            def scores_half(qsb, kb, half):
                # half 0: heads 0,1 -> scA; half 1: heads 2,3 -> scB
                r = kb - 4 * qsb
                c0 = max(r, 0) * KT
                pool = ps_scA if half == 0 else ps_scB
                sc = pool.tile([128, 2 * SB], f32, tag=f"sc{half}", name="sc")
                for hh in range(2):
                    h = 2 * half + hh
                    nc.tensor.matmul(
                        sc[:, hh * SB + c0:(hh + 1) * SB],
                        kcT[32 * h:32 * h + 32, kb * KT:(kb + 1) * KT],
                        qp[32 * h:32 * h + 32, qsb * SB + c0:(qsb + 1) * SB],
                        tile_position=(32 * h, 0),
                    )
                if half == 0:
                    es[(qsb, kb)] = ep.tile([128, 4 * SB], bf16, tag="e", name="e")
                e = es[(qsb, kb)]
                scv = sc[:].rearrange("p (h w) -> p h w", h=2)[:, :, c0:SB]
                ev = e[:].rearrange("p (h w) -> p h w", h=4)[:, 2 * half:2 * half + 2, c0:SB]
                nc.scalar.activation(ev, scv, Exp)
                if r >= 0 and half == 1:
                    # mask the diagonal 128-col block of each head (all 4)
                    ed = e[:].rearrange("p (h w) -> p h w", h=4)[:, :, c0:c0 + KT]
                    mv = mask4[:].rearrange("p (h w) -> p h w", h=4)
                    nc.vector.tensor_tensor(ed, ed, mv, mult)

            def scores_group(qsb, kb):
                scores_half(qsb, kb, 0)
                scores_half(qsb, kb, 1)

            attn_ps = {}
            den_ps = {}

            def attn_group(qsb, kb):
                r = kb - 4 * qsb
                c0 = max(r, 0) * KT
                nkb = 4 * qsb + 4
                if kb == 0:
                    attn_ps[qsb] = ps_at.tile([128, SB], f32, tag="at", name="at")
                    den_ps[qsb] = ps_dn.tile([128, SB], f32, tag="dn", name="dn")
                at, dn = attn_ps[qsb], den_ps[qsb]
                e = es.pop((qsb, kb))
                for h in range(HPC):
                    ee = e[:, h * SB + c0:(h + 1) * SB]
                    nc.tensor.matmul(
                        at[32 * h:32 * h + 32, c0:SB], vpack[kb][:, 32 * h:32 * h + 32],
                        ee, tile_position=(0, 32 * h),
                        start=(kb == 0), stop=(kb == nkb - 1),
                    )
                    nc.tensor.matmul(
                        dn[32 * h:32 * h + 1, c0:SB], ones[:, 0:1],
                        ee, tile_position=(0, 32 * h),
                        start=(kb == 0), stop=(kb == nkb - 1),
                    )

            attn_norm = {}

            def norm(qsb, lo, hi, key):
                # normalize attn columns [lo, hi) of q-block qsb
                w = hi - lo
                at, dn = attn_ps[qsb], den_ps[qsb]
                attn_sb = npo.tile([128, w], bf16, tag=f"attn_sb{w}", name="attn_sb")
                nc.vector.tensor_copy(attn_sb[:], at[:, lo:hi])
                den_sb = npo.tile([128, w], bf16, tag=f"den_sb{w}", name="den_sb")
                nc.vector.tensor_copy(den_sb[:], dn[:, lo:hi])
                # bounce den rows {32h} -> [128, w/32] for a cheap DVE reciprocal
                den_dr = dr.tile([HPC, w], bf16, tag=f"den_dr{w}", name="den_dr")
                nc.sync.dma_start(
                    den_dr[:], den_sb[:].rearrange("(h s) q -> h s q", s=32)[:, 0, :]
                )
                j = w // 32
                den_c = npo.tile([128, j], bf16, tag=f"den_c{w}", name="den_c")
                nc.sync.dma_start(
                    den_c[:], den_dr[:].rearrange("h (p j) -> (h p) j", j=j)
                )
                rec_c = npo.tile([128, j], bf16, tag=f"rec_c{w}", name="rec_c")
                with nc.allow_low_precision(reason="softmax denom recip in bf16"):
                    nc.vector.reciprocal(rec_c[:], den_c[:])
                rec_dr = dr.tile([HPC, w], bf16, tag=f"rec_dr{w}", name="rec_dr")
                nc.sync.dma_start(
                    rec_dr[:].rearrange("h (p j) -> (h p) j", j=j), rec_c[:]
                )
                recb = npo.tile([128, w], bf16, tag=f"recb{w}", name="recb")
                for h in range(HPC):
                    nc.sync.dma_start(
                        recb[32 * h:32 * h + 32, :],
                        rec_dr[h:h + 1, :].to_broadcast([32, w]),
                    )
                an = npo.tile([128, w], bf16, tag=f"an{w}", name="an")
                nc.vector.tensor_tensor(an[:], attn_sb[:], recb[:], mult)
                attn_norm[key] = (an, lo, hi)

            def proj_group(key, mb, tail=False):
                an, lo, hi = attn_norm[key]
                qsb = key if isinstance(key, int) else key[0]
                w = hi - lo
                ps = ps_big.tile([128, SB], f32, tag="big", name="psP")
                nc.tensor.matmul(ps[:, 0:w], wpj[:, mb * 128:(mb + 1) * 128], an[:])
                stage = op.tile([128, w], bf16, tag=f"stage{w}", name="stage")
                # in the tail the exps are done: route evac+DMA through the
                # idle Scalar engine for every other block to halve the chain
                if tail and mb % 2 == 1:
                    nc.scalar.copy(stage[:], ps[:, 0:w])
                    dma_eng = nc.scalar
                else:
                    nc.vector.tensor_copy(stage[:], ps[:, 0:w])
                    dma_eng = nc.sync
                dma_eng.dma_start(
                    out_t.ap()[mb * 128:(mb + 1) * 128,
                               qsb * SB + lo:qsb * SB + hi],
                    stage[:],
                )

            # ================= PE schedule =================
            def QKV(sb, mb):
                return lambda: qkv_group(sb, mb)

            def VCT(kt):
                return lambda: vc_tr(kt)

            def PRJ(key, mb):
                return lambda: proj_group(key, mb)

            def NRM3A():
                def f():
                    norm(3, 0, 256, (3, 0))
                    for mb in range(8):
                        proj_group((3, 0), mb, tail=True)
                return f

            fillers = {
                0: [[QKV(1, 0), VCT(2)], [QKV(1, 1), VCT(3)],
                    [QKV(1, 2)], []],
                1: [[QKV(2, 0)], [QKV(2, 1)], [QKV(2, 2)],
                    [VCT(4)], [VCT(5)], [VCT(6)], [VCT(7)], []],
                2: [[QKV(3, 0)], [QKV(3, 1)], [QKV(3, 2)],
                    [PRJ(0, 0)], [PRJ(0, 1)], [PRJ(0, 2), VCT(8)],
                    [PRJ(0, 3), VCT(9)], [PRJ(0, 4), VCT(10)],
                    [PRJ(0, 5), VCT(11)], [PRJ(0, 6)], [PRJ(0, 7)], []],
                3: [[PRJ(1, 0)], [PRJ(1, 1)], [PRJ(1, 2)], [PRJ(1, 3)],
                    [PRJ(1, 4)], [PRJ(1, 5)], [PRJ(1, 6)], [PRJ(1, 7)],
                    [PRJ(2, 0), VCT(12)], [PRJ(2, 1), VCT(13)],
                    [PRJ(2, 2), VCT(14)], [PRJ(2, 3), VCT(15)],
                    [PRJ(2, 4)], [PRJ(2, 5)], [PRJ(2, 6)],
                    [PRJ(2, 7), NRM3A()]],
            }

            # lead-in: q'/k_c of sb0, first scores ASAP, then v_c of sb0
            qkv_group(0, 0)
            qkv_group(0, 1)
            scores_group(0, 0)
            qkv_group(0, 2)
            vc_tr(0)
            vc_tr(1)

            for qsb in range(NSB):
                nkb = 4 * qsb + 4
                fl = fillers[qsb]
                for kb in range(nkb):
                    if not (qsb == 0 and kb == 0):
                        scores_group(qsb, kb)
                    if kb >= 1:
                        attn_group(qsb, kb - 1)
                    if kb < len(fl):
                        for thunk in fl[kb]:
                            thunk()
                attn_group(qsb, nkb - 1)
                if qsb < 3:
                    norm(qsb, 0, SB, qsb)
                    attn_ps.pop(qsb)
                    den_ps.pop(qsb)
            # tail: second half of qsb3
            norm(3, 256, SB, (3, 1))
            attn_ps.pop(3)
            den_ps.pop(3)
            for mb in range(8):
                proj_group((3, 1), mb, tail=True)

    nc.compile()
    return nc


def _prep_inputs(hidden_states, w_attn, b_attn, wk_c, wv_c, wk_d, wv_d, w_proj):
    """Per-core input maps (host-side shard + fold + pack + bf16 cast)."""
    f8 = np.float64
    hidden_T = [np.ascontiguousarray(hidden_states[b].T).astype(BF16) for b in range(B)]
    w_attn8, b_attn8 = w_attn.astype(f8), b_attn.astype(f8)
    scl = 1.0 / np.sqrt(hd)

    k = np.arange(128).reshape(128, 1)
    j = np.arange(KT).reshape(1, KT)
    mask1 = (k <= j).astype(BF16)
    mask4 = np.tile(mask1, (1, 4))
    ident = np.eye(128, dtype=BF16)

    in_maps = []
    for c in range(NCORES):
        b = c // 4
        hs = [4 * (c % 4) + h for h in range(HPC)]
        wq_cols, bq = [], []
        wk_cols, bk = [], []
        wv_cols, bv = [], []
        for h in hs:
            wq_cols.append(w_attn8[:, h * hd:(h + 1) * hd] @ wk_d[h].astype(f8).T * scl)
            bq.append(b_attn8[h * hd:(h + 1) * hd] @ wk_d[h].astype(f8).T * scl)
            wk_cols.append(w_attn8[:, D + h * hd:D + (h + 1) * hd] @ wk_c[h].astype(f8))
            bk.append(b_attn8[D + h * hd:D + (h + 1) * hd] @ wk_c[h].astype(f8))
            wv_cols.append(
                w_attn8[:, 2 * D + h * hd:2 * D + (h + 1) * hd] @ wv_c[h].astype(f8))
            bv.append(b_attn8[2 * D + h * hd:2 * D + (h + 1) * hd] @ wv_c[h].astype(f8))
        w_qkv_l = np.concatenate(
            [np.concatenate(wq_cols, 1), np.concatenate(wk_cols, 1),
             np.concatenate(wv_cols, 1)], axis=1,
        ).astype(BF16)                                          # [1024, 384]
        b_qkv_l = np.stack(
            [np.concatenate(bq), np.concatenate(bk), np.concatenate(bv)],
            axis=1,
        ).astype(np.float32)                                    # [128, 3]
        w_projp_l = np.concatenate(
            [wv_d[h].astype(f8) @ w_proj[h * hd:(h + 1) * hd, :].astype(f8)
             for h in hs], axis=0,
        ).astype(BF16)                                          # [128, 1024]
        in_maps.append(
            {
                "hidden_t": hidden_T[b],
                "w_qkv": w_qkv_l,
                "b_qkv": b_qkv_l,
                "w_projp": w_projp_l,
                "mask_in": mask4,
                "ident_in": ident,
            }
        )
    return in_maps


def kernel(
    hidden_states,
    w_attn,
    b_attn,
    w_proj,
    b_proj,
    wk_c,
    wv_c,
    wk_d,
    wv_d,
    _trace=False,
):
    from concourse.bass_utils import run_bass_kernel_spmd

    if "nc" not in _cache:
        _cache["nc"] = _build()
    nc = _cache["nc"]

    in_maps = _prep_inputs(
        np.asarray(hidden_states),
        np.asarray(w_attn),
        np.asarray(b_attn),
        np.asarray(wk_c),
        np.asarray(wv_c),
        np.asarray(wk_d),
        np.asarray(wv_d),
        np.asarray(w_proj),
    )
    res = run_bass_kernel_spmd(
        nc, in_maps, core_ids=list(range(NCORES)), trace=_trace
    )
    out = np.empty((B, S, D), np.float32)
    for b in range(B):
        acc = np.zeros((D, S), np.float32)
        for c in range(4 * b, 4 * b + 4):
            acc += res.results[c]["out_t"].astype(np.float32)
        out[b] = acc.T + np.asarray(b_proj, np.float32)
    if _trace:
        _cache["last_exec_time_ns"] = res.exec_time_ns
        _cache["last_results"] = res
    return out
